# revision 52
# baseline (speedup 1.0000x reference)
"""Multi-head attention kernel for Trainium2 (8 NeuronCores via axon).

Problem: B=2, H=16, S=2048, D=64, fp32, mask all-False.
    scores = Q @ K^T; scores/sqrt(2048); softmax; out = attn @ V

Sharding: B*H = 32 heads -> 8 cores x 4 heads (pure data parallel).

Default config: v9 (ATT_DVE=2) — HW 124.0 us, rel err 5.6e-3. v9 is v3
with ONE change: Q^T/K^T row-duplicated on the host to [128, S] = [X; X]
so each group's two QK chunk-matmuls are emitted at tile_position
(0,0)/(64,0). LDWEIGHTS then pulls ahead (different row groups) and the
two N=512 streams overlap on disjoint ingress lanes — bench_pair.py
measured 247 ns/pair vs 424 ns each at the same position. Numerics are
bit-identical to v3 (same data, different physical partitions).

The 2026-08-10 session also explored five bigger restructures, all of
which measured SLOWER on hardware than v3 (141.9 us) despite favorable
sim/model predictions (sim is ~14% optimistic for big-MM kernels and
badly under-models per-matmul overheads + LDWEIGHTS):
  - v5 transposed-AV (exp(scores) as FWL weights, V streams, no PE
    transposes): 222 us — 1024 N=68 matmuls die on ~150-200ns/MM
    framework overhead.
  - v6 DMA-transpose epilogue + stacked-pair QK: 208.8 us — 64
    transpose-DMAs serialize the queue.
  - v7 head-pair-stacked QK at tile_position (0,0)/(64,0) (+ optional
    ATT_AVQ=1 quarter-split AV, which RACES on HW: concurrent
    different-row-group MMs may not accumulate into the same PSUM bank):
    147.1 us with the QK pair adjacent, 170 us with AV sandwiched
    between the pair (full-row-group AV fences the pair apart).
  - bench_pair.py microbench: isolated row-paired MMs DO stream
    concurrently (247 ns/pair vs 424 ns each same-position), but v3's
    QK/AV/exp interleave already hides most of that cost in context.

Default config (v3, ATT_DVE=2): per head, 4 query groups of 512; 16 k-chunks
paired into 8 groups per q-group:
  - QK^T: bf16 matmuls [64x128 chunk].T @ [64x512 q] -> fp32 PSUM scores
    (S^T layout, k on partitions), [128, 1024] per 2-chunk group.
  - exp: 6 of 8 groups on ScalarE (exp activation, scale folded, bf16 out);
    2 of 8 on VectorE via a Schraudolph fast-exp producing bf16 BIT PATTERNS
    with one tensor_scalar (int16(x*A + B), A,B scaled 2^7) - splits the
    PSUM-readout work across both elementwise engines (rel err ~5.6e-3,
    gate 2e-2).
  - AV: all-bf16 matmuls, lhsT = host-packed [V | ones | zeros] chunks,
    accumulated in PSUM [80, 512]; row 64 = softmax denominator (ones trick).
    AV emission lags 2 groups so PE never stalls on an exp semaphore ahead
    of the next QK matmuls.
  - Epilogue: copy to SBUF, 4x PE identity-transpose to [q, 80] PSUM,
    one batched reciprocal of the denominators, 4x tensor_scalar multiply,
    DMA out. Deferred into the next q-group to overlap.
PSUM: score tiles 3x2 banks + AV accumulator 1 + transpose scratch 1 = 8.
"""

import math
import sys

import numpy as np

if "/opt/trn_rl_repo" not in sys.path:
    sys.path.insert(0, "/opt/trn_rl_repo")

B, H, S, D = 2, 16, 2048, 64
N_CORES = 8
H_PER = (B * H) // N_CORES  # 4 heads per core
NT = S // 128               # 16 k-chunks
QG = S // 512               # 4 query groups of 512
# k-chunk grouping per exp call: 3 chunks = 3 PSUM banks, double buffered
# (6 banks) + AV accumulator + transpose bank = 8 banks total.
def _groups(gsz):
    gs, a = [], 0
    while a < NT:
        b = min(a + gsz, NT)
        gs.append((a, b))
        a = b
    return gs
GROUPS = None  # set per-config in _build_nc
SCALE = 1.0 / math.sqrt(S)

_CACHE = {}


def _build_nc(epi="pe", qk="bf16", gsz=2, sps_bufs=3, probe="", reps=1,
              hw_loop_reps=1):
    global GROUPS
    GROUPS = _groups(gsz)
    import concourse.tile as tile
    from concourse import bacc, mybir
    from concourse.masks import make_identity
    from contextlib import ExitStack

    f32 = mybir.dt.float32
    bf16 = mybir.dt.bfloat16
    f32r = mybir.dt.float32r
    qk_dt = bf16 if qk == "bf16" else f32

    nc = bacc.Bacc("TRN2", target_bir_lowering=False, debug=False)

    qt_d = nc.dram_tensor("qt", [H_PER, D, S], qk_dt, kind="ExternalInput").ap()
    kt_d = nc.dram_tensor("kt", [H_PER, D, S], qk_dt, kind="ExternalInput").ap()
    v_d = nc.dram_tensor("v", [H_PER, S, D], f32, kind="ExternalInput").ap()
    o_d = nc.dram_tensor("out", [H_PER, S, D], f32, kind="ExternalOutput").ap()

    def mm_in(ap):
        return ap.bitcast(f32r) if qk == "f32r" else ap

    with tile.TileContext(nc) as tc, ExitStack() as ctx:
        qt_pool = ctx.enter_context(tc.tile_pool(name="qt", bufs=2))
        kt_pool = ctx.enter_context(tc.tile_pool(name="kt", bufs=2))
        v_pool = ctx.enter_context(tc.tile_pool(name="vp", bufs=2))
        vs_pool = ctx.enter_context(tc.tile_pool(name="vs", bufs=2))
        import os
        p_pool = ctx.enter_context(
            tc.tile_pool(name="pp", bufs=int(os.environ.get("ATT_PPB", "3")))
        )
        o_pool = ctx.enter_context(tc.tile_pool(name="op", bufs=2))
        r_pool = ctx.enter_context(tc.tile_pool(name="rp", bufs=3))
        res_pool = ctx.enter_context(tc.tile_pool(name="resp", bufs=2))
        sps_pool = ctx.enter_context(
            tc.tile_pool(name="sps", bufs=sps_bufs, space="PSUM")
        )
        if epi == "pe":
            av_bufs, x_bufs = 1, 0
            id_pool = ctx.enter_context(tc.tile_pool(name="idp", bufs=1))
            tps_pool = ctx.enter_context(
                tc.tile_pool(name="tps", bufs=1, space="PSUM")
            )
            ident = id_pool.tile([128, 128], f32)
            make_identity(nc, ident[:])
        else:
            av_bufs, x_bufs = 2, 3
            x_pool = ctx.enter_context(tc.tile_pool(name="xp", bufs=x_bufs))
        av_pool = ctx.enter_context(
            tc.tile_pool(name="av", bufs=av_bufs, space="PSUM")
        )

        def emit_load(h):
            # split loads so the first S^T group's inputs land fast
            qt = qt_pool.tile([D, S], qk_dt)
            kt = kt_pool.tile([D, S], qk_dt)
            nc.sync.dma_start(kt[:, 0:512], kt_d[h, :, 0:512])
            nc.sync.dma_start(qt[:, 0:512], qt_d[h, :, 0:512])
            nc.sync.dma_start(kt[:, 512:S], kt_d[h, :, 512:S])
            nc.sync.dma_start(qt[:, 512:S], qt_d[h, :, 512:S])
            # V chunks with a ones column + zero pad: [128, 16*80] (fp32r)
            vs = vs_pool.tile([128, NT * 80], f32)
            vs3 = vs[:].rearrange("p (t e) -> p t e", e=80)
            for vq in range(4):
                nc.sync.dma_start(
                    vs3[:, vq * 4:(vq + 1) * 4, 0:64],
                    v_d[h, vq * 512:(vq + 1) * 512, :].rearrange(
                        "(t p) d -> p t d", p=128
                    ),
                )
            nc.vector.memset(vs3[:, :, 64:65], 1.0)
            nc.vector.memset(vs3[:, :, 65:80], 0.0)
            vx = v_pool.tile([128, NT * 80], f32r)
            nc.vector.tensor_copy(vx[:], vs[:])
            return qt, kt, vx

        def emit_epilogue(h, qg, av):
            # out^T [80, 512] -> transpose -> divide -> out
            res = res_pool.tile([128, 4 * 64], f32)
            if epi == "pe":
                sb = o_pool.tile([80, 512], f32)
                nc.vector.tensor_copy(sb[:], av[:])
                tp = tps_pool.tile([128, 4 * 80], f32)
                for t in range(4):
                    nc.tensor.transpose(
                        tp[:, t * 80:(t + 1) * 80],
                        sb[:, t * 128:(t + 1) * 128],
                        ident[0:80, 0:80],
                    )
                    rec = r_pool.tile([128, 1], f32)
                    nc.vector.reciprocal(rec[:], tp[:, t * 80 + 64:t * 80 + 65])
                    nc.vector.tensor_scalar_mul(
                        res[:, t * 64:(t + 1) * 64],
                        tp[:, t * 80:t * 80 + 64],
                        rec[:],
                    )
            else:
                ot = o_pool.tile([80, 512], bf16)
                nc.vector.tensor_copy(ot[:], av[:])
                for t in range(4):
                    xt = x_pool.tile([128, 80], bf16)
                    nc.sync.dma_start(
                        xt[:], ot[:, t * 128:(t + 1) * 128], transpose=True
                    )
                    rec = r_pool.tile([128, 1], f32)
                    nc.vector.reciprocal(rec[:], xt[:, 64:65])
                    nc.vector.tensor_scalar_mul(
                        res[:, t * 64:(t + 1) * 64], xt[:, 0:64], rec[:]
                    )
            nc.sync.dma_start(
                o_d[h, qg * 512:(qg + 1) * 512, :].rearrange(
                    "(t p) d -> p t d", p=128
                ),
                res[:].rearrange("p (t d) -> p t d", d=64),
            )

        def emit_body():
          pending = None  # deferred epilogue: (h, qg, av)
          for h in range(H_PER):
            qt, kt, vx = emit_load(h)
            for qg in range(QG):
                av = av_pool.tile([80, 512], f32)

                def emit_st(a, b, sp):
                    for i in range(b - a):
                        kc = a + i
                        nc.tensor.matmul(
                            sp[:, i * 512:(i + 1) * 512],
                            lhsT=mm_in(kt[:, kc * 128:(kc + 1) * 128]),
                            rhs=mm_in(qt[:, qg * 512:(qg + 1) * 512]),
                            start=True,
                            stop=True,
                        )

                def emit_av(a, b, pt):
                    for i in range(b - a):
                        kc = a + i
                        if probe == "noav" and kc > 0:
                            # timing probe: only kc==0 AV matmul, results garbage
                            continue
                        nc.tensor.matmul(
                            av[:],
                            lhsT=vx[:, kc * 80:(kc + 1) * 80],
                            rhs=pt[:, i * 512:(i + 1) * 512],
                            start=(kc == 0),
                            stop=(kc == NT - 1) or probe == "noav",
                        )

                prev = None
                for gi, (a, b) in enumerate(GROUPS):
                    n = b - a
                    sp = sps_pool.tile([128, 512 * gsz], f32)
                    emit_st(a, b, sp)
                    pt = p_pool.tile([128, 512 * gsz], f32r)
                    if probe == "smallexp":
                        # timing probe: 1/4 ACT work, results garbage
                        nc.scalar.activation(
                            pt[:, : (n * 512) // 4],
                            sp[:, : (n * 512) // 4],
                            mybir.ActivationFunctionType.Exp,
                            scale=SCALE,
                        )
                    else:
                        nc.scalar.activation(
                            pt[:, : n * 512],
                            sp[:, : n * 512],
                            mybir.ActivationFunctionType.Exp,
                            scale=SCALE,
                        )
                    if gi == 1 and pending is not None:
                        emit_epilogue(*pending)
                        pending = None
                    if prev is not None:
                        emit_av(*prev)
                    prev = (a, b, pt)
                emit_av(*prev)
                pending = (h, qg, av)
          if pending is not None:
            emit_epilogue(*pending)

        if hw_loop_reps > 1:
            with tc.For_i(0, hw_loop_reps):
                emit_body()
        else:
            for rep in range(reps):
                emit_body()

    nc.compile()
    return nc


LOG2E = 1.4426950408889634


def _dve_chunks(d):
    """Spread d DVE chunks among chunk indices 2..15 (0,1 stay on ACT so the
    deferred epilogue lands between two ACT calls)."""
    if d <= 0:
        return set()
    assert d <= 14
    return {2 + round(i * 13 / max(d - 1, 1)) for i in range(d)} if d > 1 else {8}


def _build_nc2(d_dve=4, epicopy="dve", schc=0.045, probe="", reps=1,
               hw_loop_reps=1):
    """v2: q-groups paired to 1024 wide; exp split ACT (bf16 out) / DVE
    (Schraudolph int32 -> f32r); AV dtype follows the exp engine."""
    import concourse.tile as tile
    from concourse import bacc, mybir
    from concourse.masks import make_identity
    from contextlib import ExitStack

    f32 = mybir.dt.float32
    bf16 = mybir.dt.bfloat16
    f32r = mybir.dt.float32r
    i16 = mybir.dt.int16
    Alu = mybir.AluOpType

    dve_set = _dve_chunks(d_dve)
    # Schraudolph, bf16-bits variant: exp(s*SCALE) ~ bitcast_bf16(
    # int16(s*A + B)) with A,B scaled by 2^7 (bf16 has a 7-bit mantissa)
    SCHA = float(SCALE * LOG2E * (1 << 7))
    SCHB = float(127.0 * (1 << 7) - schc * (1 << 7))

    nc = bacc.Bacc("TRN2", target_bir_lowering=False, debug=False)

    qt_d = nc.dram_tensor("qt", [H_PER, D, S], bf16, kind="ExternalInput").ap()
    kt_d = nc.dram_tensor("kt", [H_PER, D, S], bf16, kind="ExternalInput").ap()
    vb_d = nc.dram_tensor("vb", [H_PER, 128, NT * 80], bf16,
                          kind="ExternalInput").ap()
    o_d = nc.dram_tensor("out", [H_PER, S, D], f32, kind="ExternalOutput").ap()

    with tile.TileContext(nc) as tc, ExitStack() as ctx:
        qt_pool = ctx.enter_context(tc.tile_pool(name="qt", bufs=2))
        kt_pool = ctx.enter_context(tc.tile_pool(name="kt", bufs=2))
        vb_pool = ctx.enter_context(tc.tile_pool(name="vb", bufs=2))
        if dve_set:
            pf_pool = ctx.enter_context(tc.tile_pool(name="pf", bufs=3))
        pb_pool = ctx.enter_context(tc.tile_pool(name="pb", bufs=3))
        o_pool = ctx.enter_context(tc.tile_pool(name="op", bufs=2))
        r_pool = ctx.enter_context(tc.tile_pool(name="rp", bufs=2))
        res_pool = ctx.enter_context(tc.tile_pool(name="resp", bufs=2))
        id_pool = ctx.enter_context(tc.tile_pool(name="idp", bufs=1))
        sps_pool = ctx.enter_context(
            tc.tile_pool(name="sps", bufs=2, space="PSUM"))
        av_pool = ctx.enter_context(
            tc.tile_pool(name="av", bufs=1, space="PSUM"))
        tps_pool = ctx.enter_context(
            tc.tile_pool(name="tps", bufs=1, space="PSUM"))

        ident = id_pool.tile([128, 128], f32)
        make_identity(nc, ident[:])

        def emit_load(h):
            qt = qt_pool.tile([D, S], bf16)
            kt = kt_pool.tile([D, S], bf16)
            nc.sync.dma_start(kt[:, 0:512], kt_d[h, :, 0:512])
            nc.sync.dma_start(qt[:, 0:1024], qt_d[h, :, 0:1024])
            nc.sync.dma_start(kt[:, 512:S], kt_d[h, :, 512:S])
            nc.sync.dma_start(qt[:, 1024:S], qt_d[h, :, 1024:S])
            vb = vb_pool.tile([128, NT * 80], bf16)
            nc.sync.dma_start(vb[:], vb_d[h])
            return qt, kt, vb

        def emit_epilogue(h, qgp, av):
            # av: out^T [80, 1024] (rows 0-63 = d, row 64 = denominator)
            sb = o_pool.tile([80, 1024], f32)
            if epicopy == "act":
                nc.scalar.copy(sb[:], av[:])
            else:
                nc.vector.tensor_copy(sb[:], av[:])
            tps = [tps_pool.tile([128, 4 * 80], f32, name=f"tp{i}")
                   for i in range(2)]
            tp3s = [tp[:].rearrange("p (t e) -> p t e", e=80) for tp in tps]
            for t in range(8):
                nc.tensor.transpose(
                    tps[t // 4][:, (t % 4) * 80:(t % 4 + 1) * 80],
                    sb[:, t * 128:(t + 1) * 128],
                    ident[0:80, 0:80],
                )
            rec = r_pool.tile([128, 8], f32)
            rec3 = rec[:].rearrange("p (t e) -> p t e", e=1)
            for half in range(2):
                nc.vector.reciprocal(
                    rec3[:, half * 4:(half + 1) * 4, :],
                    tp3s[half][:, :, 64:65])
            res = res_pool.tile([128, 8 * 64], f32)
            res3 = res[:].rearrange("p (t d) -> p t d", d=64)
            for t in range(8):
                nc.vector.tensor_scalar_mul(
                    res3[:, t, :], tp3s[t // 4][:, t % 4, 0:64],
                    rec[:, t:t + 1])
            nc.sync.dma_start(
                o_d[h, qgp * 1024:(qgp + 1) * 1024, :].rearrange(
                    "(t p) d -> p t d", p=128),
                res3,
            )

        def emit_body():
            pending = None
            for h in range(H_PER):
                qt, kt, vb = emit_load(h)
                for qgp in range(2):
                    av = av_pool.tile([80, 1024], f32)

                    def emit_av(c, pt_ap, vx_ap):
                        if probe == "noav" and c > 0:
                            return
                        for j in range(2):
                            nc.tensor.matmul(
                                av[:, j * 512:(j + 1) * 512],
                                lhsT=vx_ap[:, c * 80:(c + 1) * 80],
                                rhs=pt_ap[:, j * 512:(j + 1) * 512],
                                start=(c == 0),
                                stop=(c == NT - 1) or probe == "noav",
                            )

                    prev = None
                    for c in range(NT):
                        sp = sps_pool.tile([128, 1024], f32)
                        for j in range(2):
                            nc.tensor.matmul(
                                sp[:, j * 512:(j + 1) * 512],
                                lhsT=kt[:, c * 128:(c + 1) * 128],
                                rhs=qt[:, qgp * 1024 + j * 512:
                                       qgp * 1024 + (j + 1) * 512],
                                start=True,
                                stop=True,
                            )
                        if c in dve_set:
                            pf = pf_pool.tile([128, 1024], bf16)
                            nc.vector.tensor_scalar(
                                pf[:].bitcast(i16), sp[:], SCHA, SCHB,
                                Alu.mult, Alu.add)
                            pt_ap, vx_ap = pf[:], vb[:]
                        else:
                            pb = pb_pool.tile([128, 1024], bf16)
                            if probe == "smallexp":
                                nc.scalar.activation(
                                    pb[:, 0:256], sp[:, 0:256],
                                    mybir.ActivationFunctionType.Exp,
                                    scale=SCALE)
                            else:
                                nc.scalar.activation(
                                    pb[:], sp[:],
                                    mybir.ActivationFunctionType.Exp,
                                    scale=SCALE)
                            pt_ap, vx_ap = pb[:], vb[:]
                        if c == 1 and pending is not None:
                            emit_epilogue(*pending)
                            pending = None
                        if prev is not None:
                            emit_av(*prev)
                        prev = (c, pt_ap, vx_ap)
                    emit_av(*prev)
                    pending = (h, qgp, av)
            if pending is not None:
                emit_epilogue(*pending)

        if hw_loop_reps > 1:
            with tc.For_i(0, hw_loop_reps):
                emit_body()
        else:
            for rep in range(reps):
                emit_body()

    nc.compile()
    return nc


def _build_nc3(d_dve=2, schc=0.045, epicopy="dve", probe="", reps=1,
               hw_loop_reps=1, hw_unroll=1):
    """v3: v1 skeleton (512-q groups, sps depth 3) + bf16 AV everywhere +
    ACT/DVE exp split (DVE = int16 Schraudolph -> bf16 bits) + host-packed V
    + batched epilogue reciprocal."""
    import concourse.tile as tile
    from concourse import bacc, mybir
    from concourse.masks import make_identity
    from contextlib import ExitStack

    f32 = mybir.dt.float32
    bf16 = mybir.dt.bfloat16
    i16 = mybir.dt.int16
    Alu = mybir.AluOpType

    # group gi (2 chunks, FD=1024) -> DVE when gi in dve_gset; gi 0,1 stay
    # ACT. d_dve=7: no whole-DVE groups; instead EVERY group is split 50/50,
    # ACT on PSUM bank 0 / DVE on bank 1, concurrently (bank-aligned so the
    # engines never contend for a PSUM bank port).
    assert 0 <= d_dve <= 7
    split_mode = d_dve == 7
    if d_dve == 0 or split_mode:
        dve_gset = set()
    elif d_dve == 1:
        dve_gset = {4}
    else:
        dve_gset = {2 + round(i * 5 / (d_dve - 1)) for i in range(d_dve)}
    SCHA = float(SCALE * LOG2E * (1 << 7))
    SCHB = float(127.0 * (1 << 7) - schc * (1 << 7))

    nc = bacc.Bacc("TRN2", target_bir_lowering=False, debug=False)

    qt_d = nc.dram_tensor("qt", [H_PER, D, S], bf16, kind="ExternalInput").ap()
    kt_d = nc.dram_tensor("kt", [H_PER, D, S], bf16, kind="ExternalInput").ap()
    vb_d = nc.dram_tensor("vb", [H_PER, 128, NT * 80], bf16,
                          kind="ExternalInput").ap()
    o_d = nc.dram_tensor("out", [H_PER, S, D], f32, kind="ExternalOutput").ap()

    with tile.TileContext(nc) as tc, ExitStack() as ctx:
        qt_pool = ctx.enter_context(tc.tile_pool(name="qt", bufs=2))
        kt_pool = ctx.enter_context(tc.tile_pool(name="kt", bufs=2))
        vb_pool = ctx.enter_context(tc.tile_pool(name="vb", bufs=2))
        pb_pool = ctx.enter_context(tc.tile_pool(name="pb", bufs=6))
        if dve_gset:
            pf_pool = ctx.enter_context(tc.tile_pool(name="pf", bufs=6))
        o_pool = ctx.enter_context(tc.tile_pool(name="op", bufs=2))
        r_pool = ctx.enter_context(tc.tile_pool(name="rp", bufs=3))
        res_pool = ctx.enter_context(tc.tile_pool(name="resp", bufs=3))
        id_pool = ctx.enter_context(tc.tile_pool(name="idp", bufs=1))
        sps_pool = ctx.enter_context(
            tc.tile_pool(name="sps", bufs=3, space="PSUM"))
        av_pool = ctx.enter_context(
            tc.tile_pool(name="av", bufs=1, space="PSUM"))
        tps_pool = ctx.enter_context(
            tc.tile_pool(name="tps", bufs=1, space="PSUM"))

        ident = id_pool.tile([128, 128], f32)
        make_identity(nc, ident[:])

        def emit_load(h):
            qt = qt_pool.tile([D, S], bf16)
            kt = kt_pool.tile([D, S], bf16)
            nc.gpsimd.dma_start(kt[:, 0:512], kt_d[h, :, 0:512])
            nc.gpsimd.dma_start(qt[:, 0:512], qt_d[h, :, 0:512])
            nc.gpsimd.dma_start(kt[:, 512:S], kt_d[h, :, 512:S])
            nc.gpsimd.dma_start(qt[:, 512:S], qt_d[h, :, 512:S])
            vb = vb_pool.tile([128, NT * 80], bf16)
            nc.gpsimd.dma_start(vb[:], vb_d[h])
            return qt, kt, vb

        def emit_epilogue(h, qg, av):
            sb = o_pool.tile([80, 512], f32)
            if epicopy == "act":
                nc.scalar.copy(sb[:], av[:])
            else:
                nc.vector.tensor_copy(sb[:], av[:])
            tp = tps_pool.tile([128, 4 * 80], f32)
            tp3 = tp[:].rearrange("p (t e) -> p t e", e=80)
            for t in range(4):
                nc.tensor.transpose(
                    tp[:, t * 80:(t + 1) * 80],
                    sb[:, t * 128:(t + 1) * 128],
                    ident[0:80, 0:80],
                )
            rec = r_pool.tile([128, 4], f32)
            nc.vector.reciprocal(
                rec[:].rearrange("p (t e) -> p t e", e=1), tp3[:, :, 64:65])
            res = res_pool.tile([128, 4 * 64], f32)
            res3 = res[:].rearrange("p (t d) -> p t d", d=64)
            for t in range(4):
                nc.vector.tensor_scalar_mul(
                    res3[:, t, :], tp3[:, t, 0:64], rec[:, t:t + 1])
            nc.sync.dma_start(
                o_d[h, qg * 512:(qg + 1) * 512, :].rearrange(
                    "(t p) d -> p t d", p=128),
                res3,
            )

        def emit_body():
            pending = None
            for h in range(H_PER):
                qt, kt, vb = emit_load(h)
                for qg in range(QG):
                    av = av_pool.tile([80, 512], f32)

                    def emit_av(gi, pt_ap):
                        for i in range(2):
                            c = 2 * gi + i
                            if "noav" in probe and c > 0:
                                continue
                            nc.tensor.matmul(
                                av[:],
                                lhsT=vb[:, c * 80:(c + 1) * 80],
                                rhs=pt_ap[:, i * 512:(i + 1) * 512],
                                start=(c == 0),
                                stop=(c == NT - 1) or "noav" in probe,
                            )

                    avq = []
                    for gi in range(8):
                        sp = sps_pool.tile([128, 1024], f32)
                        qkw = 128 if "smallqk" in probe else 512
                        for i in range(2):
                            nc.tensor.matmul(
                                sp[:, i * 512:i * 512 + qkw],
                                lhsT=kt[:, (2 * gi + i) * 128:
                                        (2 * gi + i + 1) * 128],
                                rhs=qt[:, qg * 512:qg * 512 + qkw],
                                start=True,
                                stop=True,
                            )
                        if split_mode:
                            pb = pb_pool.tile([128, 1024], bf16)
                            nc.scalar.activation(
                                pb[:, 0:512], sp[:, 0:512],
                                mybir.ActivationFunctionType.Exp,
                                scale=SCALE)
                            nc.vector.tensor_scalar(
                                pb[:, 512:1024].bitcast(i16),
                                sp[:, 512:1024], SCHA, SCHB,
                                Alu.mult, Alu.add)
                            pt_ap = pb[:]
                        elif gi in dve_gset:
                            pf = pf_pool.tile([128, 1024], bf16)
                            nc.vector.tensor_scalar(
                                pf[:].bitcast(i16), sp[:], SCHA, SCHB,
                                Alu.mult, Alu.add)
                            pt_ap = pf[:]
                        else:
                            pb = pb_pool.tile([128, 1024], bf16)
                            if "smallexp" in probe:
                                nc.scalar.activation(
                                    pb[:, 0:256], sp[:, 0:256],
                                    mybir.ActivationFunctionType.Exp,
                                    scale=SCALE)
                            else:
                                nc.scalar.activation(
                                    pb[:], sp[:],
                                    mybir.ActivationFunctionType.Exp,
                                    scale=SCALE)
                            pt_ap = pb[:]
                        if gi == 1 and pending is not None:
                            emit_epilogue(*pending)
                            pending = None
                        # AV lags 2 groups so the PE stream never stalls on
                        # an exp semaphore ahead of the next QK matmuls
                        if len(avq) == 2:
                            emit_av(*avq.pop(0))
                        avq.append((gi, pt_ap))
                    for item in avq:
                        emit_av(*item)
                    avq.clear()
                    pending = (h, qg, av)
            if pending is not None:
                emit_epilogue(*pending)

        if hw_loop_reps > 1:
            with tc.For_i(0, hw_loop_reps):
                for _ in range(hw_unroll):
                    emit_body()
        else:
            for rep in range(reps):
                emit_body()

    nc.compile()
    return nc


def _build_nc9(d_dve=2, schc=0.045, epicopy="dve", avp=0, probe="", reps=1,
               hw_loop_reps=1, hw_unroll=1):
    """v9: v3 EXACTLY, except Q^T/K^T are row-duplicated on the host to
    [128, S] ([X; X]) and each group's two QK chunk-matmuls are emitted at
    tile_position (0,0) / (64,0) using the two row copies. Consecutive QK
    MMs then differ in row group, so their LDWEIGHTS pull ahead and the two
    N=512 streams overlap (bench_pair.py: 247 ns/pair vs 424 ns each at the
    same position). They write different PSUM banks (sp col halves), so no
    drain race. AV, exp split, epilogue, PSUM budget (sps 3x2 + av + tps =
    8) are untouched v3.
    """
    import concourse.tile as tile
    from concourse import bacc, mybir
    from concourse.masks import make_identity
    from contextlib import ExitStack

    f32 = mybir.dt.float32
    bf16 = mybir.dt.bfloat16
    i16 = mybir.dt.int16
    Alu = mybir.AluOpType

    assert 0 <= d_dve <= 7
    split_mode = d_dve == 7
    if d_dve == 0 or split_mode:
        dve_gset = set()
    elif d_dve == 1:
        dve_gset = {4}
    else:
        dve_gset = {2 + round(i * 5 / (d_dve - 1)) for i in range(d_dve)}
    SCHA = float(SCALE * LOG2E * (1 << 7))
    SCHB = float(127.0 * (1 << 7) - schc * (1 << 7))

    nc = bacc.Bacc("TRN2", target_bir_lowering=False, debug=False)

    qt_d = nc.dram_tensor("qt", [H_PER, 128, S], bf16,
                          kind="ExternalInput").ap()
    kt_d = nc.dram_tensor("kt", [H_PER, 128, S], bf16,
                          kind="ExternalInput").ap()
    vb_d = nc.dram_tensor("vb", [H_PER, 128, NT * 80], bf16,
                          kind="ExternalInput").ap()
    o_d = nc.dram_tensor("out", [H_PER, S, D], f32, kind="ExternalOutput").ap()

    with tile.TileContext(nc) as tc, ExitStack() as ctx:
        qt_pool = ctx.enter_context(tc.tile_pool(name="qt", bufs=2))
        kt_pool = ctx.enter_context(tc.tile_pool(name="kt", bufs=2))
        vb_pool = ctx.enter_context(tc.tile_pool(name="vb", bufs=2))
        pb_pool = ctx.enter_context(tc.tile_pool(name="pb", bufs=6))
        if dve_gset:
            pf_pool = ctx.enter_context(tc.tile_pool(name="pf", bufs=6))
        o_pool = ctx.enter_context(tc.tile_pool(name="op", bufs=2))
        r_pool = ctx.enter_context(tc.tile_pool(name="rp", bufs=3))
        res_pool = ctx.enter_context(tc.tile_pool(name="resp", bufs=3))
        id_pool = ctx.enter_context(tc.tile_pool(name="idp", bufs=1))
        # avp mode needs a second accumulator bank; pay for it with sps=2
        sps_pool = ctx.enter_context(
            tc.tile_pool(name="sps", bufs=(2 if avp else 3), space="PSUM"))
        av_pool = ctx.enter_context(
            tc.tile_pool(name="av", bufs=1, space="PSUM"))
        tps_pool = ctx.enter_context(
            tc.tile_pool(name="tps", bufs=1, space="PSUM"))

        ident = id_pool.tile([128, 128], f32)
        make_identity(nc, ident[:])
        if avp:
            so_pool = ctx.enter_context(tc.tile_pool(name="sop", bufs=2))
            idb_pool = ctx.enter_context(tc.tile_pool(name="idb", bufs=1))
            identb = idb_pool.tile([128, 128], bf16)
            nc.vector.tensor_copy(identb[:], ident[:])

        def emit_load(h):
            qt = qt_pool.tile([128, S], bf16)
            kt = kt_pool.tile([128, S], bf16)
            nc.gpsimd.dma_start(kt[:, 0:512], kt_d[h, :, 0:512])
            nc.gpsimd.dma_start(qt[:, 0:512], qt_d[h, :, 0:512])
            nc.gpsimd.dma_start(kt[:, 512:S], kt_d[h, :, 512:S])
            nc.gpsimd.dma_start(qt[:, 512:S], qt_d[h, :, 512:S])
            vb = vb_pool.tile([128, NT * 80], bf16)
            nc.gpsimd.dma_start(vb[:], vb_d[h])
            return qt, kt, vb

        def emit_epilogue(h, qg, av):
            sb = o_pool.tile([80, 512], f32)
            if epicopy == "act":
                nc.scalar.copy(sb[:], av[:])
            else:
                nc.vector.tensor_copy(sb[:], av[:])
            tp = tps_pool.tile([128, 4 * 80], f32)
            tp3 = tp[:].rearrange("p (t e) -> p t e", e=80)
            for t in range(4):
                nc.tensor.transpose(
                    tp[:, t * 80:(t + 1) * 80],
                    sb[:, t * 128:(t + 1) * 128],
                    ident[0:80, 0:80],
                )
            rec = r_pool.tile([128, 4], f32)
            nc.vector.reciprocal(
                rec[:].rearrange("p (t e) -> p t e", e=1), tp3[:, :, 64:65])
            res = res_pool.tile([128, 4 * 64], f32)
            res3 = res[:].rearrange("p (t d) -> p t d", d=64)
            for t in range(4):
                nc.vector.tensor_scalar_mul(
                    res3[:, t, :], tp3[:, t, 0:64], rec[:, t:t + 1])
            nc.sync.dma_start(
                o_d[h, qg * 512:(qg + 1) * 512, :].rearrange(
                    "(t p) d -> p t d", p=128),
                res3,
            )

        def emit_body():
            pending = None
            for h in range(H_PER):
                qt, kt, vb = emit_load(h)
                for qg in range(QG):
                    av = av_pool.tile([80, 512], f32, name="ava")
                    avo = (av_pool.tile([80, 512], f32, name="avo")
                           if avp else None)

                    def emit_av(gi, pt_ap):
                        for i in range(2):
                            c = 2 * gi + i
                            if "noav" in probe and c > 0:
                                continue
                            if avp:
                                # lo half -> av @(0,0) || hi half -> avo
                                # @(64,0): concurrent pair into different
                                # banks; chain-internal order is fenced by
                                # the same-row-group cell conflict.
                                nc.tensor.matmul(
                                    av[:],
                                    lhsT=vb[0:64, c * 80:(c + 1) * 80],
                                    rhs=pt_ap[0:64,
                                              i * 512:(i + 1) * 512],
                                    start=(c == 0), stop=False,
                                    tile_position=(0, 0),
                                )
                                nc.tensor.matmul(
                                    avo[:],
                                    lhsT=vb[64:128, c * 80:(c + 1) * 80],
                                    rhs=pt_ap[64:128,
                                              i * 512:(i + 1) * 512],
                                    start=(c == 0),
                                    stop=(c == NT - 1) or "noav" in probe,
                                    tile_position=(64, 0),
                                )
                            else:
                                nc.tensor.matmul(
                                    av[:],
                                    lhsT=vb[:, c * 80:(c + 1) * 80],
                                    rhs=pt_ap[:, i * 512:(i + 1) * 512],
                                    start=(c == 0),
                                    stop=(c == NT - 1) or "noav" in probe,
                                )

                    def emit_av_merge():
                        # avo (hi-half sums) -> bf16 SBUF -> one identity
                        # matmul accumulates onto av, closing av's group.
                        sbo = so_pool.tile([80, 512], bf16)
                        nc.vector.tensor_copy(sbo[:], avo[:])
                        nc.tensor.matmul(
                            av[:],
                            lhsT=identb[0:80, 0:80],
                            rhs=sbo[:],
                            start=False, stop=True,
                        )

                    avq = []
                    for gi in range(8):
                        sp = sps_pool.tile([128, 1024], f32)
                        qkw = 128 if "smallqk" in probe else 512
                        for i in range(2):
                            nc.tensor.matmul(
                                sp[:, i * 512:i * 512 + qkw],
                                lhsT=kt[i * 64:(i + 1) * 64,
                                        (2 * gi + i) * 128:
                                        (2 * gi + i + 1) * 128],
                                rhs=qt[i * 64:(i + 1) * 64,
                                       qg * 512:qg * 512 + qkw],
                                start=True,
                                stop=True,
                                tile_position=(i * 64, 0),
                            )
                        if gi in dve_gset:
                            pf = pf_pool.tile([128, 1024], bf16)
                            nc.vector.tensor_scalar(
                                pf[:].bitcast(i16), sp[:], SCHA, SCHB,
                                Alu.mult, Alu.add)
                            pt_ap = pf[:]
                        else:
                            pb = pb_pool.tile([128, 1024], bf16)
                            if "smallexp" in probe:
                                nc.scalar.activation(
                                    pb[:, 0:256], sp[:, 0:256],
                                    mybir.ActivationFunctionType.Exp,
                                    scale=SCALE)
                            else:
                                nc.scalar.activation(
                                    pb[:], sp[:],
                                    mybir.ActivationFunctionType.Exp,
                                    scale=SCALE)
                            pt_ap = pb[:]
                        if gi == 1 and pending is not None:
                            emit_epilogue(*pending)
                            pending = None
                        if len(avq) == 2:
                            emit_av(*avq.pop(0))
                        avq.append((gi, pt_ap))
                    for item in avq:
                        emit_av(*item)
                    avq.clear()
                    if avp:
                        emit_av_merge()
                    pending = (h, qg, av)
            if pending is not None:
                emit_epilogue(*pending)

        if hw_loop_reps > 1:
            with tc.For_i(0, hw_loop_reps):
                for _ in range(hw_unroll):
                    emit_body()
        else:
            for rep in range(reps):
                emit_body()

    nc.compile()
    return nc


def _build_nc4(d_dve=6, schc=0.045, epicopy="dve", probe="", reps=1,
               hw_loop_reps=1, hw_unroll=1):
    """v4: v3 + row-tiled head-paired QK. Heads are processed in pairs; the
    two heads' K-chunk weights occupy PE rows 0-63 / 64-127 (tile_position
    (0,0) / (64,0)) and stream concurrently, nearly halving QK PE time.
    Score tile [128, 1024] = chunk c for head A (cols 0:512) + head B
    (cols 512:1024); one FD=1024 exp call covers both."""
    import concourse.tile as tile
    from concourse import bacc, mybir
    from concourse.masks import make_identity
    from contextlib import ExitStack

    f32 = mybir.dt.float32
    bf16 = mybir.dt.bfloat16
    i16 = mybir.dt.int16
    Alu = mybir.AluOpType

    dve_set = _dve_chunks(d_dve)
    SCHA = float(SCALE * LOG2E * (1 << 7))
    SCHB = float(127.0 * (1 << 7) - schc * (1 << 7))

    nc = bacc.Bacc("TRN2", target_bir_lowering=False, debug=False)

    # head-pair layout: qt2[hp] = [128, S] with head 2hp on partitions 0-63,
    # head 2hp+1 on 64-127 (plain reshape of the [4, 64, S] layout)
    qt_d = nc.dram_tensor("qt", [H_PER // 2, 128, S], bf16,
                          kind="ExternalInput").ap()
    kt_d = nc.dram_tensor("kt", [H_PER // 2, 128, S], bf16,
                          kind="ExternalInput").ap()
    vb_d = nc.dram_tensor("vb", [H_PER, 128, NT * 80], bf16,
                          kind="ExternalInput").ap()
    o_d = nc.dram_tensor("out", [H_PER, S, D], f32, kind="ExternalOutput").ap()

    with tile.TileContext(nc) as tc, ExitStack() as ctx:
        qt_pool = ctx.enter_context(tc.tile_pool(name="qt", bufs=2))
        kt_pool = ctx.enter_context(tc.tile_pool(name="kt", bufs=2))
        vb_pool = ctx.enter_context(tc.tile_pool(name="vb", bufs=2))
        pb_pool = ctx.enter_context(tc.tile_pool(name="pb", bufs=4))
        if dve_set:
            pf_pool = ctx.enter_context(tc.tile_pool(name="pf", bufs=4))
        o_pool = ctx.enter_context(tc.tile_pool(name="op", bufs=2))
        r_pool = ctx.enter_context(tc.tile_pool(name="rp", bufs=2))
        res_pool = ctx.enter_context(tc.tile_pool(name="resp", bufs=2))
        id_pool = ctx.enter_context(tc.tile_pool(name="idp", bufs=1))
        sps_pool = ctx.enter_context(
            tc.tile_pool(name="sps", bufs=2, space="PSUM"))
        av_pool = ctx.enter_context(
            tc.tile_pool(name="av", bufs=1, space="PSUM"))
        tps_pool = ctx.enter_context(
            tc.tile_pool(name="tps", bufs=2, space="PSUM"))

        ident = id_pool.tile([128, 128], f32)
        make_identity(nc, ident[:])

        def emit_load(hp):
            qt = qt_pool.tile([128, S], bf16)
            kt = kt_pool.tile([128, S], bf16)
            nc.sync.dma_start(kt[:, 0:512], kt_d[hp, :, 0:512])
            nc.sync.dma_start(qt[:, 0:512], qt_d[hp, :, 0:512])
            nc.sync.dma_start(kt[:, 512:S], kt_d[hp, :, 512:S])
            nc.sync.dma_start(qt[:, 512:S], qt_d[hp, :, 512:S])
            vba = vb_pool.tile([128, NT * 80], bf16, name="vba")
            vbb = vb_pool.tile([128, NT * 80], bf16, name="vbb")
            nc.sync.dma_start(vba[:], vb_d[2 * hp])
            nc.sync.dma_start(vbb[:], vb_d[2 * hp + 1])
            return qt, kt, vba, vbb

        def emit_epilogue(h, qg, av):
            sb = o_pool.tile([80, 512], f32)
            if epicopy == "act":
                nc.scalar.copy(sb[:], av[:])
            else:
                nc.vector.tensor_copy(sb[:], av[:])
            tp = tps_pool.tile([128, 4 * 80], f32)
            tp3 = tp[:].rearrange("p (t e) -> p t e", e=80)
            for t in range(4):
                nc.tensor.transpose(
                    tp[:, t * 80:(t + 1) * 80],
                    sb[:, t * 128:(t + 1) * 128],
                    ident[0:80, 0:80],
                )
            rec = r_pool.tile([128, 4], f32)
            nc.vector.reciprocal(
                rec[:].rearrange("p (t e) -> p t e", e=1), tp3[:, :, 64:65])
            res = res_pool.tile([128, 4 * 64], f32)
            res3 = res[:].rearrange("p (t d) -> p t d", d=64)
            for t in range(4):
                nc.vector.tensor_scalar_mul(
                    res3[:, t, :], tp3[:, t, 0:64], rec[:, t:t + 1])
            nc.sync.dma_start(
                o_d[h, qg * 512:(qg + 1) * 512, :].rearrange(
                    "(t p) d -> p t d", p=128),
                res3,
            )

        def emit_body():
            pending = []
            for hp in range(H_PER // 2):
                qt, kt, vba, vbb = emit_load(hp)
                for qg in range(QG):
                    ava = av_pool.tile([80, 512], f32, name="ava")
                    avb = av_pool.tile([80, 512], f32, name="avb")

                    def emit_av(c, pt_ap):
                        if "noav" in probe and c > 0:
                            return
                        stop = (c == NT - 1) or "noav" in probe
                        nc.tensor.matmul(
                            ava[:],
                            lhsT=vba[:, c * 80:(c + 1) * 80],
                            rhs=pt_ap[:, 0:512],
                            start=(c == 0), stop=stop,
                        )
                        nc.tensor.matmul(
                            avb[:],
                            lhsT=vbb[:, c * 80:(c + 1) * 80],
                            rhs=pt_ap[:, 512:1024],
                            start=(c == 0), stop=stop,
                        )

                    avq = []
                    for c in range(NT):
                        sp = sps_pool.tile([128, 1024], f32)
                        qkw = 128 if "smallqk" in probe else 512
                        for half in range(2):
                            nc.tensor.matmul(
                                sp[:, half * 512:half * 512 + qkw],
                                lhsT=kt[half * 64:(half + 1) * 64,
                                        c * 128:(c + 1) * 128],
                                rhs=qt[half * 64:(half + 1) * 64,
                                       qg * 512:qg * 512 + qkw],
                                start=True,
                                stop=True,
                                tile_position=(half * 64, 0),
                            )
                        if c in dve_set:
                            pf = pf_pool.tile([128, 1024], bf16)
                            nc.vector.tensor_scalar(
                                pf[:].bitcast(i16), sp[:], SCHA, SCHB,
                                Alu.mult, Alu.add)
                            pt_ap = pf[:]
                        else:
                            pb = pb_pool.tile([128, 1024], bf16)
                            if "smallexp" in probe:
                                nc.scalar.activation(
                                    pb[:, 0:256], sp[:, 0:256],
                                    mybir.ActivationFunctionType.Exp,
                                    scale=SCALE)
                            else:
                                nc.scalar.activation(
                                    pb[:], sp[:],
                                    mybir.ActivationFunctionType.Exp,
                                    scale=SCALE)
                            pt_ap = pb[:]
                        if c in (1, 2) and pending:
                            emit_epilogue(*pending.pop(0))
                        if len(avq) == 2:
                            emit_av(*avq.pop(0))
                        avq.append((c, pt_ap))
                    for item in avq:
                        emit_av(*item)
                    avq.clear()
                    pending = [(2 * hp, qg, ava), (2 * hp + 1, qg, avb)]
            for item in pending:
                emit_epilogue(*item)

        if hw_loop_reps > 1:
            with tc.For_i(0, hw_loop_reps):
                for _ in range(hw_unroll):
                    emit_body()
        else:
            for rep in range(reps):
                emit_body()

    nc.compile()
    return nc


def _build_nc5(d_dve=13, schc=0.045, epimul="act", sps_bufs=2, probe="", reps=1,
               hw_loop_reps=1, hw_unroll=1):
    """v5: transposed-AV ("AVT") attention.

    Per head pair (2 heads stacked on partitions 0-63 / 64-127):
      - QK^T per (qg, kc): two matmuls (tile_position (0,0)/(64,0)) into two
        1-bank fp32 PSUM score tiles spA/spB [128 k, 512 q].
      - exp: 32 calls per (pair, qg) of FD=512, split ACT (exp activation)
        vs DVE (Schraudolph int16 bf16-bits) by slot -> p tiles [128, 512]
        bf16 in SBUF (k on partitions, q on free).
      - AVT: p tile is the STATIONARY operand ([128 k, 128 q-block] weights,
        FWL-eligible), V chunk [128 k, 68] streams (cols 0:64 V, 64 ones,
        65:68 zero) -> out [128 q, 68] f32 accumulated over all 16 k-chunks
        in PSUM; col 64 accumulates the softmax denominator. No PE transpose
        epilogue at all; out is already [q, d].
      - 4 q-block accumulator chains share one PSUM bank per head; since
        start=True clears has_written for the WHOLE bank, the bank is
        pre-zeroed by one [1x128]@[1x272] zero matmul and all chains run
        start=False.
      - Epilogue per (head, qg): reciprocal of col 64 + 4 per-partition
        scalar muls (ACT or DVE) -> [128, 4*64] SBUF -> DMA.
    PSUM: spA/spB (sps_bufs each) + avA/avB (2 each) = 2*sps_bufs + 4 banks.
    """
    import concourse.tile as tile
    from concourse import bacc, mybir
    from contextlib import ExitStack

    f32 = mybir.dt.float32
    bf16 = mybir.dt.bfloat16
    i16 = mybir.dt.int16
    Alu = mybir.AluOpType

    # slot = 2*c + head (0..31); slots 0,1 (c=0) stay ACT.
    assert 0 <= d_dve <= 30
    dve_set = set()
    if d_dve == 1:
        dve_set = {16}
    elif d_dve > 1:
        dve_set = {2 + round(i * 29 / (d_dve - 1)) for i in range(d_dve)}
        # fill any rounding collisions
        j = 2
        while len(dve_set) < d_dve:
            if j not in dve_set:
                dve_set.add(j)
            j += 1
    SCHA = float(SCALE * LOG2E * (1 << 7))
    SCHB = float(127.0 * (1 << 7) - schc * (1 << 7))

    nc = bacc.Bacc("TRN2", target_bir_lowering=False, debug=False)

    qt_d = nc.dram_tensor("qt", [H_PER // 2, 128, S], bf16,
                          kind="ExternalInput").ap()
    kt_d = nc.dram_tensor("kt", [H_PER // 2, 128, S], bf16,
                          kind="ExternalInput").ap()
    vb_d = nc.dram_tensor("vb", [H_PER, 128, NT * 68], bf16,
                          kind="ExternalInput").ap()
    o_d = nc.dram_tensor("out", [H_PER, S, D], f32, kind="ExternalOutput").ap()

    with tile.TileContext(nc) as tc, ExitStack() as ctx:
        qt_pool = ctx.enter_context(tc.tile_pool(name="qt", bufs=2))
        kt_pool = ctx.enter_context(tc.tile_pool(name="kt", bufs=2))
        vb_pool = ctx.enter_context(tc.tile_pool(name="vb", bufs=2))
        p_pool = ctx.enter_context(tc.tile_pool(name="pp", bufs=4))
        z_pool = ctx.enter_context(tc.tile_pool(name="zp", bufs=1))
        r_pool = ctx.enter_context(tc.tile_pool(name="rp", bufs=3))
        res_pool = ctx.enter_context(tc.tile_pool(name="resp", bufs=3))
        sps_pool = ctx.enter_context(
            tc.tile_pool(name="sps", bufs=sps_bufs, space="PSUM"))
        av_pool = ctx.enter_context(
            tc.tile_pool(name="av", bufs=2, space="PSUM"))

        zt = z_pool.tile([1, 512], bf16)
        nc.vector.memset(zt[:], 0.0)

        def emit_load(hp):
            qt = qt_pool.tile([128, S], bf16)
            kt = kt_pool.tile([128, S], bf16)
            nc.sync.dma_start(kt[:, 0:512], kt_d[hp, :, 0:512])
            nc.sync.dma_start(qt[:, 0:512], qt_d[hp, :, 0:512])
            vba = vb_pool.tile([128, NT * 68], bf16, name="vba")
            vbb = vb_pool.tile([128, NT * 68], bf16, name="vbb")
            nc.sync.dma_start(vba[:], vb_d[2 * hp])
            nc.sync.dma_start(vbb[:], vb_d[2 * hp + 1])
            nc.sync.dma_start(kt[:, 512:S], kt_d[hp, :, 512:S])
            nc.sync.dma_start(qt[:, 512:S], qt_d[hp, :, 512:S])
            return qt, kt, vba, vbb

        def emit_epilogue(h, qg, av):
            av3 = av[:].rearrange("p (t e) -> p t e", e=68)
            rec = r_pool.tile([128, 4], f32)
            nc.vector.reciprocal(
                rec[:].rearrange("p (t e) -> p t e", e=1), av3[:, :, 64:65])
            res = res_pool.tile([128, 4 * 64], f32)
            res3 = res[:].rearrange("p (t d) -> p t d", d=64)
            for t in range(4):
                on_act = epimul == "act" or (epimul == "mix" and t % 2 == 0)
                if on_act:
                    nc.scalar.mul(res3[:, t, :], av3[:, t, 0:64],
                                  rec[:, t:t + 1])
                else:
                    nc.vector.tensor_scalar_mul(
                        res3[:, t, :], av3[:, t, 0:64], rec[:, t:t + 1])
            nc.sync.dma_start(
                o_d[h, qg * 512:(qg + 1) * 512, :].rearrange(
                    "(t p) d -> p t d", p=128),
                res3,
            )

        def emit_body():
            pending = []
            for hp in range(H_PER // 2):
                qt, kt, vba, vbb = emit_load(hp)
                for qg in range(QG):
                    avA = av_pool.tile([128, 4 * 68], f32, name="avA")
                    avB = av_pool.tile([128, 4 * 68], f32, name="avB")
                    # pre-zero the accumulator banks (sets has_written so the
                    # start=False chains below accumulate onto 0)
                    for av in (avA, avB):
                        nc.tensor.matmul(
                            av[:], lhsT=zt[0:1, 0:128], rhs=zt[0:1, 0:272],
                            start=True, stop=False, skip_group_check=True)

                    def emit_avt(c, p_ap, av, vb):
                        for qb in range(4):
                            if "noav" in probe and c > 0:
                                continue
                            nc.tensor.matmul(
                                av[:, qb * 68:qb * 68 + 68],
                                lhsT=p_ap[:, qb * 128:(qb + 1) * 128],
                                rhs=vb[:, c * 68:(c + 1) * 68],
                                start=False,
                                stop=(c == NT - 1) or "noav" in probe,
                                skip_group_check=True,
                            )

                    def emit_exp(c, head, sp):
                        p = p_pool.tile([128, 512], bf16,
                                        name=f"p{'ab'[head]}")
                        w = 128 if "smallexp" in probe else 512
                        if (2 * c + head) in dve_set:
                            nc.vector.tensor_scalar(
                                p[:, 0:w].bitcast(i16), sp[:, 0:w],
                                SCHA, SCHB, Alu.mult, Alu.add)
                        else:
                            nc.scalar.activation(
                                p[:, 0:w], sp[:, 0:w],
                                mybir.ActivationFunctionType.Exp,
                                scale=SCALE)
                        return p

                    avq = []
                    for c in range(NT):
                        qkw = 128 if "smallqk" in probe else 512
                        item = avq.pop(0) if len(avq) == 2 else None
                        if item is not None:
                            emit_avt(item[0], item[1], avA, vba)
                        spA = sps_pool.tile([128, 512], f32, name="spA")
                        nc.tensor.matmul(
                            spA[:, 0:qkw],
                            lhsT=kt[0:64, c * 128:(c + 1) * 128],
                            rhs=qt[0:64, qg * 512:qg * 512 + qkw],
                            start=True, stop=True, tile_position=(0, 0))
                        if item is not None:
                            emit_avt(item[0], item[2], avB, vbb)
                        spB = sps_pool.tile([128, 512], f32, name="spB")
                        nc.tensor.matmul(
                            spB[:, 0:qkw],
                            lhsT=kt[64:128, c * 128:(c + 1) * 128],
                            rhs=qt[64:128, qg * 512:qg * 512 + qkw],
                            start=True, stop=True, tile_position=(64, 0))
                        pA = emit_exp(c, 0, spA)
                        pB = emit_exp(c, 1, spB)
                        if c in (1, 2) and pending:
                            emit_epilogue(*pending.pop(0))
                        avq.append((c, pA, pB))
                    for item in avq:
                        emit_avt(item[0], item[1], avA, vba)
                        emit_avt(item[0], item[2], avB, vbb)
                    avq.clear()
                    pending = [(2 * hp, qg, avA), (2 * hp + 1, qg, avB)]
            for item in pending:
                emit_epilogue(*item)

        if hw_loop_reps > 1:
            with tc.For_i(0, hw_loop_reps):
                for _ in range(hw_unroll):
                    emit_body()
        else:
            for rep in range(reps):
                emit_body()

    nc.compile()
    return nc


def _build_nc6(d_dve=8, schc=0.045, epimul="mix", epicopy="dve", probe="",
               reps=1, hw_loop_reps=1, hw_unroll=1):
    """v6: v3 flow with (a) head-pair-stacked QK (two row-tiled matmuls at
    tile_position (0,0)/(64,0) share one [128, 1024] score tile = [A | B]),
    (b) DMA-transpose epilogue (no PE transposes, no tps PSUM bank: out^T
    [80,512] -> bf16 SBUF -> 4 transpose-DMAs -> [128, 4*80] -> recip + 4
    per-partition muls, bf16 res, host upcasts), (c) exp split ACT/DVE by
    k-chunk (FD=1024 calls), epilogue ops mixed across ACT/DVE.
    PSUM: sps 3x2 banks + avA + avB = 8.
    """
    import concourse.tile as tile
    from concourse import bacc, mybir
    from contextlib import ExitStack

    f32 = mybir.dt.float32
    bf16 = mybir.dt.bfloat16
    i16 = mybir.dt.int16
    Alu = mybir.AluOpType

    assert 0 <= d_dve <= 14
    if d_dve == 0:
        dve_set = set()
    elif d_dve == 1:
        dve_set = {8}
    else:
        dve_set = {2 + round(i * 13 / (d_dve - 1)) for i in range(d_dve)}
        j = 2
        while len(dve_set) < d_dve:
            if j not in dve_set:
                dve_set.add(j)
            j += 1
    SCHA = float(SCALE * LOG2E * (1 << 7))
    SCHB = float(127.0 * (1 << 7) - schc * (1 << 7))

    nc = bacc.Bacc("TRN2", target_bir_lowering=False, debug=False)

    qt_d = nc.dram_tensor("qt", [H_PER // 2, 128, S], bf16,
                          kind="ExternalInput").ap()
    kt_d = nc.dram_tensor("kt", [H_PER // 2, 128, S], bf16,
                          kind="ExternalInput").ap()
    vb_d = nc.dram_tensor("vb", [H_PER, 128, NT * 80], bf16,
                          kind="ExternalInput").ap()
    o_d = nc.dram_tensor("out", [H_PER, S, D], bf16, kind="ExternalOutput").ap()

    with tile.TileContext(nc) as tc, ExitStack() as ctx:
        qt_pool = ctx.enter_context(tc.tile_pool(name="qt", bufs=2))
        kt_pool = ctx.enter_context(tc.tile_pool(name="kt", bufs=2))
        vb_pool = ctx.enter_context(tc.tile_pool(name="vb", bufs=2))
        pb_pool = ctx.enter_context(tc.tile_pool(name="pb", bufs=6))
        o_pool = ctx.enter_context(tc.tile_pool(name="op", bufs=2))
        x_pool = ctx.enter_context(tc.tile_pool(name="xp", bufs=2))
        r_pool = ctx.enter_context(tc.tile_pool(name="rp", bufs=3))
        res_pool = ctx.enter_context(tc.tile_pool(name="resp", bufs=3))
        sps_pool = ctx.enter_context(
            tc.tile_pool(name="sps", bufs=3, space="PSUM"))
        av_pool = ctx.enter_context(
            tc.tile_pool(name="av", bufs=1, space="PSUM"))

        def emit_load(hp):
            qt = qt_pool.tile([128, S], bf16)
            kt = kt_pool.tile([128, S], bf16)
            nc.gpsimd.dma_start(kt[:, 0:512], kt_d[hp, :, 0:512])
            nc.gpsimd.dma_start(qt[:, 0:512], qt_d[hp, :, 0:512])
            vba = vb_pool.tile([128, NT * 80], bf16, name="vba")
            vbb = vb_pool.tile([128, NT * 80], bf16, name="vbb")
            nc.gpsimd.dma_start(vba[:], vb_d[2 * hp])
            nc.gpsimd.dma_start(vbb[:], vb_d[2 * hp + 1])
            nc.gpsimd.dma_start(kt[:, 512:S], kt_d[hp, :, 512:S])
            nc.gpsimd.dma_start(qt[:, 512:S], qt_d[hp, :, 512:S])
            return qt, kt, vba, vbb

        def emit_epilogue(h, qg, av):
            # av: out^T [80, 512] fp32 PSUM (row 64 = denominator)
            ot = o_pool.tile([80, 512], bf16)
            if epicopy == "act":
                nc.scalar.copy(ot[:], av[:])
            else:
                nc.vector.tensor_copy(ot[:], av[:])
            xt = x_pool.tile([128, 4 * 80], bf16)
            xt3 = xt[:].rearrange("p (t e) -> p t e", e=80)
            for t in range(4):
                nc.sync.dma_start(
                    xt3[:, t, :], ot[:, t * 128:(t + 1) * 128], transpose=True)
            rec = r_pool.tile([128, 4], f32)
            nc.vector.reciprocal(
                rec[:].rearrange("p (t e) -> p t e", e=1), xt3[:, :, 64:65])
            res = res_pool.tile([128, 4 * 64], bf16)
            res3 = res[:].rearrange("p (t d) -> p t d", d=64)
            for t in range(4):
                if epimul == "pool":
                    nc.gpsimd.tensor_scalar_mul(
                        res3[:, t, :], xt3[:, t, 0:64], rec[:, t:t + 1])
                elif epimul == "act" or (epimul == "mix" and t % 2 == 0):
                    nc.scalar.mul(res3[:, t, :], xt3[:, t, 0:64],
                                  rec[:, t:t + 1])
                else:
                    nc.vector.tensor_scalar_mul(
                        res3[:, t, :], xt3[:, t, 0:64], rec[:, t:t + 1])
            nc.sync.dma_start(
                o_d[h, qg * 512:(qg + 1) * 512, :].rearrange(
                    "(t p) d -> p t d", p=128),
                res3,
            )

        def emit_body():
            pending = []
            for hp in range(H_PER // 2):
                qt, kt, vba, vbb = emit_load(hp)
                for qg in range(QG):
                    ava = av_pool.tile([80, 512], f32, name="ava")
                    avb = av_pool.tile([80, 512], f32, name="avb")

                    def emit_av(c, pt_ap):
                        if "noav" in probe and c > 0:
                            return
                        stop = (c == NT - 1) or "noav" in probe
                        nc.tensor.matmul(
                            ava[:],
                            lhsT=vba[:, c * 80:(c + 1) * 80],
                            rhs=pt_ap[:, 0:512],
                            start=(c == 0), stop=stop,
                        )
                        nc.tensor.matmul(
                            avb[:],
                            lhsT=vbb[:, c * 80:(c + 1) * 80],
                            rhs=pt_ap[:, 512:1024],
                            start=(c == 0), stop=stop,
                        )

                    avq = []
                    for c in range(NT):
                        sp = sps_pool.tile([128, 1024], f32)
                        qkw = 128 if "smallqk" in probe else 512
                        for half in range(2):
                            nc.tensor.matmul(
                                sp[:, half * 512:half * 512 + qkw],
                                lhsT=kt[half * 64:(half + 1) * 64,
                                        c * 128:(c + 1) * 128],
                                rhs=qt[half * 64:(half + 1) * 64,
                                       qg * 512:qg * 512 + qkw],
                                start=True,
                                stop=True,
                                tile_position=(half * 64, 0),
                            )
                        pb = pb_pool.tile([128, 1024], bf16)
                        if c in dve_set:
                            nc.vector.tensor_scalar(
                                pb[:].bitcast(i16), sp[:], SCHA, SCHB,
                                Alu.mult, Alu.add)
                        else:
                            w = 256 if "smallexp" in probe else 1024
                            nc.scalar.activation(
                                pb[:, 0:w], sp[:, 0:w],
                                mybir.ActivationFunctionType.Exp,
                                scale=SCALE)
                        if c in (1, 2) and pending:
                            emit_epilogue(*pending.pop(0))
                        if len(avq) == 2:
                            emit_av(*avq.pop(0))
                        avq.append((c, pb[:]))
                    for item in avq:
                        emit_av(*item)
                    avq.clear()
                    pending = [(2 * hp, qg, ava), (2 * hp + 1, qg, avb)]
            for item in pending:
                emit_epilogue(*item)

        if hw_loop_reps > 1:
            with tc.For_i(0, hw_loop_reps):
                for _ in range(hw_unroll):
                    emit_body()
        else:
            for rep in range(reps):
                emit_body()

    nc.compile()
    return nc


def _build_nc7(d_dve=7, schc=0.045, avq=0, probe="", reps=1,
               hw_loop_reps=1, hw_unroll=1):
    """v7: fully row-group-alternating PE stream (microbench-validated: MMs
    at tile_position (0,0)/(64,0) stream CONCURRENTLY ~247ns/pair, while
    same-position MMs serialize at ~424ns each because LDWEIGHTS only pulls
    ahead when the row group differs).

      - QK: head-pair stacked, (0,0)/(64,0) -> sp [128, 1024] = [A | B].
      - AV: each chunk split into two 64-contraction halves, paired ACROSS
        heads so concurrent MMs hit different banks: slot1 = [A_lo@(0,0) ||
        B_hi@(64,0)], slot2 = [A_hi@(64,0) || B_lo@(0,0)]. Each head's
        halves land serially in ONE accumulator bank -> full sums, 2 banks.
      - exp: FD=1024 per chunk, ACT/DVE split by chunk (d_dve of 16 to DVE
        via Schraudolph bf16-bits).
      - Epilogue (on-device, v3-style): bf16 copy -> 4 PE transposes (bf16,
        into a bf16 PSUM tile) -> batched reciprocal -> 4 per-partition
        muls (all-16-bit, 2x DVE mode; mixed ACT/DVE) -> bf16 res -> DMA;
        host upcasts to f32.
    PSUM: sp (2 bufs x 2 banks) + avA + avB + tps (2 bufs x 1) = 8.
    """
    import concourse.tile as tile
    from concourse import bacc, mybir
    from contextlib import ExitStack

    f32 = mybir.dt.float32
    bf16 = mybir.dt.bfloat16
    i16 = mybir.dt.int16
    Alu = mybir.AluOpType

    assert 0 <= d_dve <= 14
    if d_dve == 0:
        dve_set = set()
    elif d_dve == 1:
        dve_set = {8}
    else:
        dve_set = {2 + round(i * 13 / (d_dve - 1)) for i in range(d_dve)}
        j = 2
        while len(dve_set) < d_dve:
            if j not in dve_set:
                dve_set.add(j)
            j += 1
    SCHA = float(SCALE * LOG2E * (1 << 7))
    SCHB = float(127.0 * (1 << 7) - schc * (1 << 7))
    avq_split = bool(avq)

    nc = bacc.Bacc("TRN2", target_bir_lowering=False, debug=False)

    qt_d = nc.dram_tensor("qt", [H_PER // 2, 128, S], bf16,
                          kind="ExternalInput").ap()
    kt_d = nc.dram_tensor("kt", [H_PER // 2, 128, S], bf16,
                          kind="ExternalInput").ap()
    vb_d = nc.dram_tensor("vb", [H_PER, 128, NT * 80], bf16,
                          kind="ExternalInput").ap()
    o_d = nc.dram_tensor("out", [H_PER, S, D], f32, kind="ExternalOutput").ap()

    with tile.TileContext(nc) as tc, ExitStack() as ctx:
        from concourse.masks import make_identity

        qt_pool = ctx.enter_context(tc.tile_pool(name="qt", bufs=2))
        kt_pool = ctx.enter_context(tc.tile_pool(name="kt", bufs=2))
        vb_pool = ctx.enter_context(tc.tile_pool(name="vb", bufs=2))
        pb_pool = ctx.enter_context(tc.tile_pool(name="pb", bufs=6))
        o_pool = ctx.enter_context(tc.tile_pool(name="op", bufs=2))
        r_pool = ctx.enter_context(tc.tile_pool(name="rp", bufs=3))
        res_pool = ctx.enter_context(tc.tile_pool(name="resp", bufs=3))
        id_pool = ctx.enter_context(tc.tile_pool(name="idp", bufs=1))
        sps_pool = ctx.enter_context(
            tc.tile_pool(name="sps", bufs=2, space="PSUM"))
        av_pool = ctx.enter_context(
            tc.tile_pool(name="av", bufs=1, space="PSUM"))
        tps_pool = ctx.enter_context(
            tc.tile_pool(name="tps", bufs=2, space="PSUM"))

        ident = id_pool.tile([128, 128], f32)
        make_identity(nc, ident[:])

        def emit_load(hp):
            qt = qt_pool.tile([128, S], bf16)
            kt = kt_pool.tile([128, S], bf16)
            nc.gpsimd.dma_start(kt[:, 0:512], kt_d[hp, :, 0:512])
            nc.gpsimd.dma_start(qt[:, 0:512], qt_d[hp, :, 0:512])
            vba = vb_pool.tile([128, NT * 80], bf16, name="vba")
            vbb = vb_pool.tile([128, NT * 80], bf16, name="vbb")
            nc.gpsimd.dma_start(vba[:], vb_d[2 * hp])
            nc.gpsimd.dma_start(vbb[:], vb_d[2 * hp + 1])
            nc.gpsimd.dma_start(kt[:, 512:S], kt_d[hp, :, 512:S])
            nc.gpsimd.dma_start(qt[:, 512:S], qt_d[hp, :, 512:S])
            return qt, kt, vba, vbb

        def emit_epilogue(h, qg, av):
            sb = o_pool.tile([80, 512], f32)
            nc.vector.tensor_copy(sb[:], av[:])
            tp = tps_pool.tile([128, 4 * 80], f32)
            tp3 = tp[:].rearrange("p (t e) -> p t e", e=80)
            for t in range(4):
                nc.tensor.transpose(
                    tp[:, t * 80:(t + 1) * 80],
                    sb[:, t * 128:(t + 1) * 128],
                    ident[0:80, 0:80],
                )
            rec = r_pool.tile([128, 4], f32)
            nc.vector.reciprocal(
                rec[:].rearrange("p (t e) -> p t e", e=1), tp3[:, :, 64:65])
            res = res_pool.tile([128, 4 * 64], f32)
            res3 = res[:].rearrange("p (t d) -> p t d", d=64)
            for t in range(4):
                nc.vector.tensor_scalar_mul(
                    res3[:, t, :], tp3[:, t, 0:64], rec[:, t:t + 1])
            nc.sync.dma_start(
                o_d[h, qg * 512:(qg + 1) * 512, :].rearrange(
                    "(t p) d -> p t d", p=128),
                res3,
            )

        def emit_body():
            pending = []
            for hp in range(H_PER // 2):
                qt, kt, vba, vbb = emit_load(hp)
                for qg in range(QG):
                    ava = av_pool.tile([80, 512], f32, name="ava")
                    avb = av_pool.tile([80, 512], f32, name="avb")

                    # AV chunk = four 64-contraction quarter-MMs, emitted in
                    # a row-group-alternating chain interleaved with the QK
                    # pair. Consecutive MMs always differ in row group (so
                    # LDWEIGHTS pulls ahead / streams overlap) AND in target
                    # PSUM bank (so concurrent drains never race); same-bank
                    # MMs are 3 apart, fenced by the same-row-group
                    # serialization in between.
                    def av_mm(av, vb, hf, col, c, start, stop):
                        if "noav" in probe and c > 0:
                            return
                        nc.tensor.matmul(
                            av[:],
                            lhsT=vb[hf * 64:(hf + 1) * 64,
                                    c * 80:(c + 1) * 80],
                            rhs=pt_refs[c][hf * 64:(hf + 1) * 64,
                                           col * 512:(col + 1) * 512],
                            start=start, stop=stop,
                            tile_position=(hf * 64, 0),
                        )

                    def av_full(av, vb, col, c, start, stop):
                        # v3-style full-contraction AV matmul at (0,0)
                        if "noav" in probe and c > 0:
                            return
                        nc.tensor.matmul(
                            av[:],
                            lhsT=vb[:, c * 80:(c + 1) * 80],
                            rhs=pt_refs[c][:, col * 512:(col + 1) * 512],
                            start=start, stop=stop,
                        )

                    pt_refs = {}
                    avq = []
                    for c in range(NT):
                        prev = avq.pop(0) if len(avq) == 2 else None
                        nav = "noav" in probe
                        if prev is not None:
                            pstop = (prev == NT - 1) or nav
                            if avq_split:
                                av_mm(ava, vba, 0, 0, prev,
                                      prev == 0, False)      # A-lo @(0,0)
                                av_mm(avb, vbb, 1, 1, prev,
                                      prev == 0, False)      # B-hi @(64,0)
                            else:
                                # both AV matmuls BEFORE the QK pair: AV uses
                                # all row groups, so placing one between QK_A
                                # and QK_B would fence the pair apart.
                                av_full(ava, vba, 0, prev, prev == 0, pstop)
                                av_full(avb, vbb, 1, prev, prev == 0, pstop)
                        sp = sps_pool.tile([128, 1024], f32)
                        qkw = 128 if "smallqk" in probe else 512
                        nc.tensor.matmul(
                            sp[:, 0:qkw],
                            lhsT=kt[0:64, c * 128:(c + 1) * 128],
                            rhs=qt[0:64, qg * 512:qg * 512 + qkw],
                            start=True, stop=True,
                            tile_position=(0, 0),
                        )
                        if prev is not None and avq_split:
                            pstop = (prev == NT - 1) or nav
                            av_mm(ava, vba, 1, 0, prev,
                                  False, pstop)              # A-hi @(64,0)
                            av_mm(avb, vbb, 0, 1, prev,
                                  False, pstop)              # B-lo @(0,0)
                        nc.tensor.matmul(
                            sp[:, 512:512 + qkw],
                            lhsT=kt[64:128, c * 128:(c + 1) * 128],
                            rhs=qt[64:128, qg * 512:qg * 512 + qkw],
                            start=True, stop=True,
                            tile_position=(64, 0),
                        )
                        pb = pb_pool.tile([128, 1024], bf16)
                        if c in dve_set:
                            nc.vector.tensor_scalar(
                                pb[:].bitcast(i16), sp[:], SCHA, SCHB,
                                Alu.mult, Alu.add)
                        else:
                            w = 256 if "smallexp" in probe else 1024
                            nc.scalar.activation(
                                pb[:, 0:w], sp[:, 0:w],
                                mybir.ActivationFunctionType.Exp,
                                scale=SCALE)
                        if c in (1, 2) and pending:
                            emit_epilogue(*pending.pop(0))
                        pt_refs[c] = pb[:]
                        avq.append(c)
                    for prev in avq:
                        nav = "noav" in probe
                        pstop = (prev == NT - 1) or nav
                        if avq_split:
                            av_mm(ava, vba, 0, 0, prev, prev == 0, False)
                            av_mm(avb, vbb, 1, 1, prev, prev == 0, False)
                            av_mm(ava, vba, 1, 0, prev, False, pstop)
                            av_mm(avb, vbb, 0, 1, prev, False, pstop)
                        else:
                            av_full(ava, vba, 0, prev, prev == 0, pstop)
                            av_full(avb, vbb, 1, prev, prev == 0, pstop)
                    avq.clear()
                    pending = [(2 * hp, qg, ava), (2 * hp + 1, qg, avb)]
            for item in pending:
                emit_epilogue(*item)

        if hw_loop_reps > 1:
            with tc.For_i(0, hw_loop_reps):
                for _ in range(hw_unroll):
                    emit_body()
        else:
            for rep in range(reps):
                emit_body()

    nc.compile()
    return nc


def _cfg():
    import os

    if os.environ.get("ATT_V9", "1") == "1":
        return (
            "v9",
            int(os.environ.get("ATT_DVE", "2")),
            float(os.environ.get("ATT_SCHC", "0.045")),
            os.environ.get("ATT_EPICOPY", "dve"),
            int(os.environ.get("ATT_AVP", "0")),
            os.environ.get("ATT_PROBE", ""),
        )
    if os.environ.get("ATT_V7", "0") == "1":
        return (
            "v7",
            int(os.environ.get("ATT_DVE", "7")),
            float(os.environ.get("ATT_SCHC", "0.045")),
            int(os.environ.get("ATT_AVQ", "0")),
            os.environ.get("ATT_PROBE", ""),
        )
    if os.environ.get("ATT_V6", "0") == "1":
        return (
            "v6",
            int(os.environ.get("ATT_DVE", "8")),
            float(os.environ.get("ATT_SCHC", "0.045")),
            os.environ.get("ATT_EPIMUL", "mix"),
            os.environ.get("ATT_EPICOPY", "dve"),
            os.environ.get("ATT_PROBE", ""),
        )
    if os.environ.get("ATT_V5", "0") == "1":
        return (
            "v5",
            int(os.environ.get("ATT_DVE", "13")),
            float(os.environ.get("ATT_SCHC", "0.045")),
            os.environ.get("ATT_EPIMUL", "act"),
            int(os.environ.get("ATT_SPSB", "2")),
            os.environ.get("ATT_PROBE", ""),
        )
    if os.environ.get("ATT_V4", "0") == "1":
        return (
            "v4",
            int(os.environ.get("ATT_DVE", "6")),
            float(os.environ.get("ATT_SCHC", "0.045")),
            os.environ.get("ATT_EPICOPY", "dve"),
            os.environ.get("ATT_PROBE", ""),
        )
    if os.environ.get("ATT_V3", "1") == "1":
        return (
            "v3",
            int(os.environ.get("ATT_DVE", "2")),
            float(os.environ.get("ATT_SCHC", "0.045")),
            os.environ.get("ATT_EPICOPY", "dve"),
            os.environ.get("ATT_PROBE", ""),
        )
    if os.environ.get("ATT_V2", "1") == "1":
        return (
            "v2",
            int(os.environ.get("ATT_DVE", "4")),
            os.environ.get("ATT_EPICOPY", "dve"),
            float(os.environ.get("ATT_SCHC", "0.045")),
            os.environ.get("ATT_PROBE", ""),
        )
    return (
        os.environ.get("ATT_EPI", "pe"),
        os.environ.get("ATT_QK", "bf16"),
        int(os.environ.get("ATT_GSZ", "2")),
        int(os.environ.get("ATT_SPSB", "3")),
        os.environ.get("ATT_PROBE", ""),
    )


def _build(cfg, **kw):
    if cfg[0] == "v9":
        return _build_nc9(*cfg[1:], **kw)
    if cfg[0] == "v7":
        return _build_nc7(*cfg[1:], **kw)
    if cfg[0] == "v6":
        return _build_nc6(*cfg[1:], **kw)
    if cfg[0] == "v5":
        return _build_nc5(*cfg[1:], **kw)
    if cfg[0] == "v4":
        return _build_nc4(*cfg[1:], **kw)
    if cfg[0] == "v3":
        return _build_nc3(*cfg[1:], **kw)
    if cfg[0] == "v2":
        return _build_nc2(*cfg[1:], **kw)
    return _build_nc(*cfg, **kw)


def _get_nc():
    cfg = _cfg()
    if cfg not in _CACHE:
        _CACHE[cfg] = _build(cfg)
    return _CACHE[cfg]


def _prep_in_maps(Q, K, V):
    import ml_dtypes

    cfg = _cfg()
    is_v2 = cfg[0] in ("v2", "v3", "v4", "v5", "v6", "v7", "v9")
    qk = "bf16" if is_v2 else cfg[1]
    tdt = ml_dtypes.bfloat16 if qk == "bf16" else np.float32
    Qr = np.ascontiguousarray(np.asarray(Q, dtype=np.float32)).reshape(B * H, S, D)
    Kr = np.ascontiguousarray(np.asarray(K, dtype=np.float32)).reshape(B * H, S, D)
    Vr = np.ascontiguousarray(np.asarray(V, dtype=np.float32)).reshape(B * H, S, D)
    # host-side layout prep: [BH, S, D] -> [BH, D, S]
    QT = np.ascontiguousarray(Qr.transpose(0, 2, 1)).astype(tdt)
    KT = np.ascontiguousarray(Kr.transpose(0, 2, 1)).astype(tdt)
    if is_v2:
        # packed V chunks [BH, 128, NT, E] = [V | ones | zeros]
        VE = 68 if cfg[0] == "v5" else 80
        # (v6 uses the classic 80-wide V-weights layout)
        vp = np.zeros((B * H, 128, NT, VE), dtype=np.float32)
        vp[:, :, :, 0:64] = Vr.reshape(B * H, NT, 128, D).transpose(0, 2, 1, 3)
        vp[:, :, :, 64] = 1.0
        vp = vp.reshape(B * H, 128, NT * VE)
        vpb = vp.astype(ml_dtypes.bfloat16)
    in_maps = []
    for c in range(N_CORES):
        sl = slice(c * H_PER, (c + 1) * H_PER)
        if is_v2:
            qtc = np.ascontiguousarray(QT[sl])
            ktc = np.ascontiguousarray(KT[sl])
            if cfg[0] in ("v4", "v5", "v6", "v7"):
                qtc = qtc.reshape(H_PER // 2, 128, S)
                ktc = ktc.reshape(H_PER // 2, 128, S)
            elif cfg[0] == "v9":
                # row-duplicate [64, S] -> [128, S] = [X; X] per head so QK
                # chunk MMs can alternate tile_position (0,0)/(64,0)
                qtc = np.ascontiguousarray(
                    np.concatenate([qtc, qtc], axis=1))
                ktc = np.ascontiguousarray(
                    np.concatenate([ktc, ktc], axis=1))
            m = {
                "qt": qtc,
                "kt": ktc,
                "vb": np.ascontiguousarray(vpb[sl]),
            }
        else:
            m = {
                "qt": np.ascontiguousarray(QT[sl]),
                "kt": np.ascontiguousarray(KT[sl]),
                "v": np.ascontiguousarray(Vr[sl]),
            }
        in_maps.append(m)
    return in_maps


def _gather(results):
    out = np.concatenate([np.asarray(r["out"]) for r in results], axis=0)
    return out.reshape(B, H, S, D).astype(np.float32)


def _numpy_fallback(Q, K, V, mask):
    # generic masked path (not used by the benchmark inputs: mask is all-False)
    Qf = np.asarray(Q, dtype=np.float64)
    Kf = np.asarray(K, dtype=np.float64)
    Vf = np.asarray(V, dtype=np.float64)
    out = np.empty((B, H, S, D), dtype=np.float32)
    for b in range(B):
        for h in range(H):
            s = Qf[b, h] @ Kf[b, h].T
            s = np.where(mask, -1e10, s) / math.sqrt(S)
            s -= s.max(axis=-1, keepdims=True)
            e = np.exp(s)
            p = e / e.sum(axis=-1, keepdims=True)
            out[b, h] = (p @ Vf[b, h]).astype(np.float32)
    return out


def _get_runner():
    """Build the sharded jit callable once; reuse across kernel() calls."""
    key = ("runner",) + _cfg()
    if key in _CACHE:
        return _CACHE[key]
    import jax
    from jax.sharding import Mesh, PartitionSpec, NamedSharding
    from jax.experimental.shard_map import shard_map
    from concourse import bass2jax, mybir
    from concourse.bass2jax import _bass_exec_p, install_neuronx_cc_hook

    nc = _get_nc()
    install_neuronx_cc_hook()
    devices = jax.devices()[:N_CORES]
    assert len(devices) == N_CORES
    mesh = Mesh(np.asarray(devices), ("core",))

    part_name = nc.partition_id_tensor.name if nc.partition_id_tensor else None
    in_names, out_names, out_avals, out_shapes = [], [], [], []
    for alloc in nc.m.functions[0].allocations:
        if not isinstance(alloc, mybir.MemoryLocationSet):
            continue
        name = alloc.memorylocations[0].name
        if alloc.kind == "ExternalInput":
            if name != part_name:
                in_names.append(name)
        elif alloc.kind == "ExternalOutput":
            out_names.append(name)
            shape = tuple(alloc.tensor_shape)
            dtype = mybir.dt.np(alloc.dtype)
            out_avals.append(jax.core.ShapedArray(shape, dtype))
            out_shapes.append((shape, dtype))
    all_names = in_names + out_names + ([part_name] if part_name else [])

    def _body(*args):
        operands = list(args)
        if part_name is not None:
            operands.append(bass2jax.partition_id_tensor())
        return tuple(
            _bass_exec_p.bind(
                *operands,
                out_avals=tuple(out_avals),
                in_names=tuple(all_names),
                out_names=tuple(out_names),
                lowering_input_output_aliases=(),
                sim_require_finite=True,
                sim_require_nnan=True,
                nc=nc,
            )
        )

    nio = len(in_names) + len(out_names)
    fn = jax.jit(
        shard_map(
            _body,
            mesh=mesh,
            in_specs=(PartitionSpec("core"),) * nio,
            out_specs=(PartitionSpec("core"),) * len(out_names),
            check_rep=False,
        ),
        keep_unused=True,
    )
    sh = NamedSharding(mesh, PartitionSpec("core"))

    def run(in_maps):
        import jax as _jax

        concat_in = [
            _jax.device_put(
                np.concatenate(
                    [np.ascontiguousarray(m[nm]) for m in in_maps], axis=0
                ),
                sh,
            )
            for nm in in_names
        ]
        concat_zeros = [
            _jax.device_put(np.zeros((N_CORES * s[0], *s[1:]), dt), sh)
            for (s, dt) in out_shapes
        ]
        outs = fn(*concat_in, *concat_zeros)
        outs = [np.asarray(o) for o in outs]
        return [
            {
                nm: outs[i].reshape(N_CORES, *out_avals[i].shape)[c]
                for i, nm in enumerate(out_names)
            }
            for c in range(N_CORES)
        ]

    _CACHE[key] = run
    return run


def run_on_device(Q, K, V, trace=False, **trace_kwargs):
    """Compile (cached) + run on the 8 cores. Returns (full_output, results)."""
    in_maps = _prep_in_maps(Q, K, V)
    if trace:
        from concourse.bass_utils import run_bass_kernel_spmd

        nc = _get_nc()
        res = run_bass_kernel_spmd(
            nc, in_maps, list(range(N_CORES)), trace=True, **trace_kwargs
        )
        return _gather(res.results), res
    results = _get_runner()(in_maps)
    return _gather(results), None


def kernel(Q, K, V, mask):
    mask = np.asarray(mask)
    if mask.any():
        return _numpy_fallback(Q, K, V, mask)
    out, _ = run_on_device(Q, K, V, trace=False)
    return out



# revision 53
# speedup vs baseline: 1.1476x; 1.1476x over previous
"""Multi-head attention kernel for Trainium2 (8 NeuronCores via axon).

Problem: B=2, H=16, S=2048, D=64, fp32, mask all-False.
    scores = Q @ K^T; scores/sqrt(2048); softmax; out = attn @ V

Sharding: B*H = 32 heads -> 8 cores x 4 heads (pure data parallel).

Default config: v9 (ATT_DVE=2) — HW 124.0 us, rel err 5.6e-3. v9 is v3
with ONE change: Q^T/K^T row-duplicated on the host to [128, S] = [X; X]
so each group's two QK chunk-matmuls are emitted at tile_position
(0,0)/(64,0). LDWEIGHTS then pulls ahead (different row groups) and the
two N=512 streams overlap on disjoint ingress lanes — bench_pair.py
measured 247 ns/pair vs 424 ns each at the same position. Numerics are
bit-identical to v3 (same data, different physical partitions).

The 2026-08-10 session also explored five bigger restructures, all of
which measured SLOWER on hardware than v3 (141.9 us) despite favorable
sim/model predictions (sim is ~14% optimistic for big-MM kernels and
badly under-models per-matmul overheads + LDWEIGHTS):
  - v5 transposed-AV (exp(scores) as FWL weights, V streams, no PE
    transposes): 222 us — 1024 N=68 matmuls die on ~150-200ns/MM
    framework overhead.
  - v6 DMA-transpose epilogue + stacked-pair QK: 208.8 us — 64
    transpose-DMAs serialize the queue.
  - v7 head-pair-stacked QK at tile_position (0,0)/(64,0) (+ optional
    ATT_AVQ=1 quarter-split AV, which RACES on HW: concurrent
    different-row-group MMs may not accumulate into the same PSUM bank):
    147.1 us with the QK pair adjacent, 170 us with AV sandwiched
    between the pair (full-row-group AV fences the pair apart).
  - bench_pair.py microbench: isolated row-paired MMs DO stream
    concurrently (247 ns/pair vs 424 ns each same-position), but v3's
    QK/AV/exp interleave already hides most of that cost in context.
  - v9 ATT_AVP=1 (AV split into lo/hi 64-contraction halves paired into
    two accumulator banks + identity-matmul merge, sps 3->2): 142.3 us —
    each half still streams all 512 columns, so the pair only saves LDW
    overlap while sps=2 stalls and the merge cost dominate. Row-pairing
    pays only when it fuses two DIFFERENT matmuls' streams (QK dup-row),
    not when splitting one matmul's contraction.

Default config (v3, ATT_DVE=2): per head, 4 query groups of 512; 16 k-chunks
paired into 8 groups per q-group:
  - QK^T: bf16 matmuls [64x128 chunk].T @ [64x512 q] -> fp32 PSUM scores
    (S^T layout, k on partitions), [128, 1024] per 2-chunk group.
  - exp: 6 of 8 groups on ScalarE (exp activation, scale folded, bf16 out);
    2 of 8 on VectorE via a Schraudolph fast-exp producing bf16 BIT PATTERNS
    with one tensor_scalar (int16(x*A + B), A,B scaled 2^7) - splits the
    PSUM-readout work across both elementwise engines (rel err ~5.6e-3,
    gate 2e-2).
  - AV: all-bf16 matmuls, lhsT = host-packed [V | ones | zeros] chunks,
    accumulated in PSUM [80, 512]; row 64 = softmax denominator (ones trick).
    AV emission lags 2 groups so PE never stalls on an exp semaphore ahead
    of the next QK matmuls.
  - Epilogue: copy to SBUF, 4x PE identity-transpose to [q, 80] PSUM,
    one batched reciprocal of the denominators, 4x tensor_scalar multiply,
    DMA out. Deferred into the next q-group to overlap.
PSUM: score tiles 3x2 banks + AV accumulator 1 + transpose scratch 1 = 8.
"""

import math
import sys

import numpy as np

if "/opt/trn_rl_repo" not in sys.path:
    sys.path.insert(0, "/opt/trn_rl_repo")

B, H, S, D = 2, 16, 2048, 64
N_CORES = 8
H_PER = (B * H) // N_CORES  # 4 heads per core
NT = S // 128               # 16 k-chunks
QG = S // 512               # 4 query groups of 512
# k-chunk grouping per exp call: 3 chunks = 3 PSUM banks, double buffered
# (6 banks) + AV accumulator + transpose bank = 8 banks total.
def _groups(gsz):
    gs, a = [], 0
    while a < NT:
        b = min(a + gsz, NT)
        gs.append((a, b))
        a = b
    return gs
GROUPS = None  # set per-config in _build_nc
SCALE = 1.0 / math.sqrt(S)

_CACHE = {}


def _build_nc(epi="pe", qk="bf16", gsz=2, sps_bufs=3, probe="", reps=1,
              hw_loop_reps=1):
    global GROUPS
    GROUPS = _groups(gsz)
    import concourse.tile as tile
    from concourse import bacc, mybir
    from concourse.masks import make_identity
    from contextlib import ExitStack

    f32 = mybir.dt.float32
    bf16 = mybir.dt.bfloat16
    f32r = mybir.dt.float32r
    qk_dt = bf16 if qk == "bf16" else f32

    nc = bacc.Bacc("TRN2", target_bir_lowering=False, debug=False)

    qt_d = nc.dram_tensor("qt", [H_PER, D, S], qk_dt, kind="ExternalInput").ap()
    kt_d = nc.dram_tensor("kt", [H_PER, D, S], qk_dt, kind="ExternalInput").ap()
    v_d = nc.dram_tensor("v", [H_PER, S, D], f32, kind="ExternalInput").ap()
    o_d = nc.dram_tensor("out", [H_PER, S, D], f32, kind="ExternalOutput").ap()

    def mm_in(ap):
        return ap.bitcast(f32r) if qk == "f32r" else ap

    with tile.TileContext(nc) as tc, ExitStack() as ctx:
        qt_pool = ctx.enter_context(tc.tile_pool(name="qt", bufs=2))
        kt_pool = ctx.enter_context(tc.tile_pool(name="kt", bufs=2))
        v_pool = ctx.enter_context(tc.tile_pool(name="vp", bufs=2))
        vs_pool = ctx.enter_context(tc.tile_pool(name="vs", bufs=2))
        import os
        p_pool = ctx.enter_context(
            tc.tile_pool(name="pp", bufs=int(os.environ.get("ATT_PPB", "3")))
        )
        o_pool = ctx.enter_context(tc.tile_pool(name="op", bufs=2))
        r_pool = ctx.enter_context(tc.tile_pool(name="rp", bufs=3))
        res_pool = ctx.enter_context(tc.tile_pool(name="resp", bufs=2))
        sps_pool = ctx.enter_context(
            tc.tile_pool(name="sps", bufs=sps_bufs, space="PSUM")
        )
        if epi == "pe":
            av_bufs, x_bufs = 1, 0
            id_pool = ctx.enter_context(tc.tile_pool(name="idp", bufs=1))
            tps_pool = ctx.enter_context(
                tc.tile_pool(name="tps", bufs=1, space="PSUM")
            )
            ident = id_pool.tile([128, 128], f32)
            make_identity(nc, ident[:])
        else:
            av_bufs, x_bufs = 2, 3
            x_pool = ctx.enter_context(tc.tile_pool(name="xp", bufs=x_bufs))
        av_pool = ctx.enter_context(
            tc.tile_pool(name="av", bufs=av_bufs, space="PSUM")
        )

        def emit_load(h):
            # split loads so the first S^T group's inputs land fast
            qt = qt_pool.tile([D, S], qk_dt)
            kt = kt_pool.tile([D, S], qk_dt)
            nc.sync.dma_start(kt[:, 0:512], kt_d[h, :, 0:512])
            nc.sync.dma_start(qt[:, 0:512], qt_d[h, :, 0:512])
            nc.sync.dma_start(kt[:, 512:S], kt_d[h, :, 512:S])
            nc.sync.dma_start(qt[:, 512:S], qt_d[h, :, 512:S])
            # V chunks with a ones column + zero pad: [128, 16*80] (fp32r)
            vs = vs_pool.tile([128, NT * 80], f32)
            vs3 = vs[:].rearrange("p (t e) -> p t e", e=80)
            for vq in range(4):
                nc.sync.dma_start(
                    vs3[:, vq * 4:(vq + 1) * 4, 0:64],
                    v_d[h, vq * 512:(vq + 1) * 512, :].rearrange(
                        "(t p) d -> p t d", p=128
                    ),
                )
            nc.vector.memset(vs3[:, :, 64:65], 1.0)
            nc.vector.memset(vs3[:, :, 65:80], 0.0)
            vx = v_pool.tile([128, NT * 80], f32r)
            nc.vector.tensor_copy(vx[:], vs[:])
            return qt, kt, vx

        def emit_epilogue(h, qg, av):
            # out^T [80, 512] -> transpose -> divide -> out
            res = res_pool.tile([128, 4 * 64], f32)
            if epi == "pe":
                sb = o_pool.tile([80, 512], f32)
                nc.vector.tensor_copy(sb[:], av[:])
                tp = tps_pool.tile([128, 4 * 80], f32)
                for t in range(4):
                    nc.tensor.transpose(
                        tp[:, t * 80:(t + 1) * 80],
                        sb[:, t * 128:(t + 1) * 128],
                        ident[0:80, 0:80],
                    )
                    rec = r_pool.tile([128, 1], f32)
                    nc.vector.reciprocal(rec[:], tp[:, t * 80 + 64:t * 80 + 65])
                    nc.vector.tensor_scalar_mul(
                        res[:, t * 64:(t + 1) * 64],
                        tp[:, t * 80:t * 80 + 64],
                        rec[:],
                    )
            else:
                ot = o_pool.tile([80, 512], bf16)
                nc.vector.tensor_copy(ot[:], av[:])
                for t in range(4):
                    xt = x_pool.tile([128, 80], bf16)
                    nc.sync.dma_start(
                        xt[:], ot[:, t * 128:(t + 1) * 128], transpose=True
                    )
                    rec = r_pool.tile([128, 1], f32)
                    nc.vector.reciprocal(rec[:], xt[:, 64:65])
                    nc.vector.tensor_scalar_mul(
                        res[:, t * 64:(t + 1) * 64], xt[:, 0:64], rec[:]
                    )
            nc.sync.dma_start(
                o_d[h, qg * 512:(qg + 1) * 512, :].rearrange(
                    "(t p) d -> p t d", p=128
                ),
                res[:].rearrange("p (t d) -> p t d", d=64),
            )

        def emit_body():
          pending = None  # deferred epilogue: (h, qg, av)
          for h in range(H_PER):
            qt, kt, vx = emit_load(h)
            for qg in range(QG):
                av = av_pool.tile([80, 512], f32)

                def emit_st(a, b, sp):
                    for i in range(b - a):
                        kc = a + i
                        nc.tensor.matmul(
                            sp[:, i * 512:(i + 1) * 512],
                            lhsT=mm_in(kt[:, kc * 128:(kc + 1) * 128]),
                            rhs=mm_in(qt[:, qg * 512:(qg + 1) * 512]),
                            start=True,
                            stop=True,
                        )

                def emit_av(a, b, pt):
                    for i in range(b - a):
                        kc = a + i
                        if probe == "noav" and kc > 0:
                            # timing probe: only kc==0 AV matmul, results garbage
                            continue
                        nc.tensor.matmul(
                            av[:],
                            lhsT=vx[:, kc * 80:(kc + 1) * 80],
                            rhs=pt[:, i * 512:(i + 1) * 512],
                            start=(kc == 0),
                            stop=(kc == NT - 1) or probe == "noav",
                        )

                prev = None
                for gi, (a, b) in enumerate(GROUPS):
                    n = b - a
                    sp = sps_pool.tile([128, 512 * gsz], f32)
                    emit_st(a, b, sp)
                    pt = p_pool.tile([128, 512 * gsz], f32r)
                    if probe == "smallexp":
                        # timing probe: 1/4 ACT work, results garbage
                        nc.scalar.activation(
                            pt[:, : (n * 512) // 4],
                            sp[:, : (n * 512) // 4],
                            mybir.ActivationFunctionType.Exp,
                            scale=SCALE,
                        )
                    else:
                        nc.scalar.activation(
                            pt[:, : n * 512],
                            sp[:, : n * 512],
                            mybir.ActivationFunctionType.Exp,
                            scale=SCALE,
                        )
                    if gi == 1 and pending is not None:
                        emit_epilogue(*pending)
                        pending = None
                    if prev is not None:
                        emit_av(*prev)
                    prev = (a, b, pt)
                emit_av(*prev)
                pending = (h, qg, av)
          if pending is not None:
            emit_epilogue(*pending)

        if hw_loop_reps > 1:
            with tc.For_i(0, hw_loop_reps):
                emit_body()
        else:
            for rep in range(reps):
                emit_body()

    nc.compile()
    return nc


LOG2E = 1.4426950408889634


def _dve_chunks(d):
    """Spread d DVE chunks among chunk indices 2..15 (0,1 stay on ACT so the
    deferred epilogue lands between two ACT calls)."""
    if d <= 0:
        return set()
    assert d <= 14
    return {2 + round(i * 13 / max(d - 1, 1)) for i in range(d)} if d > 1 else {8}


def _build_nc2(d_dve=4, epicopy="dve", schc=0.045, probe="", reps=1,
               hw_loop_reps=1):
    """v2: q-groups paired to 1024 wide; exp split ACT (bf16 out) / DVE
    (Schraudolph int32 -> f32r); AV dtype follows the exp engine."""
    import concourse.tile as tile
    from concourse import bacc, mybir
    from concourse.masks import make_identity
    from contextlib import ExitStack

    f32 = mybir.dt.float32
    bf16 = mybir.dt.bfloat16
    f32r = mybir.dt.float32r
    i16 = mybir.dt.int16
    Alu = mybir.AluOpType

    dve_set = _dve_chunks(d_dve)
    # Schraudolph, bf16-bits variant: exp(s*SCALE) ~ bitcast_bf16(
    # int16(s*A + B)) with A,B scaled by 2^7 (bf16 has a 7-bit mantissa)
    SCHA = float(SCALE * LOG2E * (1 << 7))
    SCHB = float(127.0 * (1 << 7) - schc * (1 << 7))

    nc = bacc.Bacc("TRN2", target_bir_lowering=False, debug=False)

    qt_d = nc.dram_tensor("qt", [H_PER, D, S], bf16, kind="ExternalInput").ap()
    kt_d = nc.dram_tensor("kt", [H_PER, D, S], bf16, kind="ExternalInput").ap()
    vb_d = nc.dram_tensor("vb", [H_PER, 128, NT * 80], bf16,
                          kind="ExternalInput").ap()
    o_d = nc.dram_tensor("out", [H_PER, S, D], f32, kind="ExternalOutput").ap()

    with tile.TileContext(nc) as tc, ExitStack() as ctx:
        qt_pool = ctx.enter_context(tc.tile_pool(name="qt", bufs=2))
        kt_pool = ctx.enter_context(tc.tile_pool(name="kt", bufs=2))
        vb_pool = ctx.enter_context(tc.tile_pool(name="vb", bufs=2))
        if dve_set:
            pf_pool = ctx.enter_context(tc.tile_pool(name="pf", bufs=3))
        pb_pool = ctx.enter_context(tc.tile_pool(name="pb", bufs=3))
        o_pool = ctx.enter_context(tc.tile_pool(name="op", bufs=2))
        r_pool = ctx.enter_context(tc.tile_pool(name="rp", bufs=2))
        res_pool = ctx.enter_context(tc.tile_pool(name="resp", bufs=2))
        id_pool = ctx.enter_context(tc.tile_pool(name="idp", bufs=1))
        sps_pool = ctx.enter_context(
            tc.tile_pool(name="sps", bufs=2, space="PSUM"))
        av_pool = ctx.enter_context(
            tc.tile_pool(name="av", bufs=1, space="PSUM"))
        tps_pool = ctx.enter_context(
            tc.tile_pool(name="tps", bufs=1, space="PSUM"))

        ident = id_pool.tile([128, 128], f32)
        make_identity(nc, ident[:])

        def emit_load(h):
            qt = qt_pool.tile([D, S], bf16)
            kt = kt_pool.tile([D, S], bf16)
            nc.sync.dma_start(kt[:, 0:512], kt_d[h, :, 0:512])
            nc.sync.dma_start(qt[:, 0:1024], qt_d[h, :, 0:1024])
            nc.sync.dma_start(kt[:, 512:S], kt_d[h, :, 512:S])
            nc.sync.dma_start(qt[:, 1024:S], qt_d[h, :, 1024:S])
            vb = vb_pool.tile([128, NT * 80], bf16)
            nc.sync.dma_start(vb[:], vb_d[h])
            return qt, kt, vb

        def emit_epilogue(h, qgp, av):
            # av: out^T [80, 1024] (rows 0-63 = d, row 64 = denominator)
            sb = o_pool.tile([80, 1024], f32)
            if epicopy == "act":
                nc.scalar.copy(sb[:], av[:])
            else:
                nc.vector.tensor_copy(sb[:], av[:])
            tps = [tps_pool.tile([128, 4 * 80], f32, name=f"tp{i}")
                   for i in range(2)]
            tp3s = [tp[:].rearrange("p (t e) -> p t e", e=80) for tp in tps]
            for t in range(8):
                nc.tensor.transpose(
                    tps[t // 4][:, (t % 4) * 80:(t % 4 + 1) * 80],
                    sb[:, t * 128:(t + 1) * 128],
                    ident[0:80, 0:80],
                )
            rec = r_pool.tile([128, 8], f32)
            rec3 = rec[:].rearrange("p (t e) -> p t e", e=1)
            for half in range(2):
                nc.vector.reciprocal(
                    rec3[:, half * 4:(half + 1) * 4, :],
                    tp3s[half][:, :, 64:65])
            res = res_pool.tile([128, 8 * 64], f32)
            res3 = res[:].rearrange("p (t d) -> p t d", d=64)
            for t in range(8):
                nc.vector.tensor_scalar_mul(
                    res3[:, t, :], tp3s[t // 4][:, t % 4, 0:64],
                    rec[:, t:t + 1])
            nc.sync.dma_start(
                o_d[h, qgp * 1024:(qgp + 1) * 1024, :].rearrange(
                    "(t p) d -> p t d", p=128),
                res3,
            )

        def emit_body():
            pending = None
            for h in range(H_PER):
                qt, kt, vb = emit_load(h)
                for qgp in range(2):
                    av = av_pool.tile([80, 1024], f32)

                    def emit_av(c, pt_ap, vx_ap):
                        if probe == "noav" and c > 0:
                            return
                        for j in range(2):
                            nc.tensor.matmul(
                                av[:, j * 512:(j + 1) * 512],
                                lhsT=vx_ap[:, c * 80:(c + 1) * 80],
                                rhs=pt_ap[:, j * 512:(j + 1) * 512],
                                start=(c == 0),
                                stop=(c == NT - 1) or probe == "noav",
                            )

                    prev = None
                    for c in range(NT):
                        sp = sps_pool.tile([128, 1024], f32)
                        for j in range(2):
                            nc.tensor.matmul(
                                sp[:, j * 512:(j + 1) * 512],
                                lhsT=kt[:, c * 128:(c + 1) * 128],
                                rhs=qt[:, qgp * 1024 + j * 512:
                                       qgp * 1024 + (j + 1) * 512],
                                start=True,
                                stop=True,
                            )
                        if c in dve_set:
                            pf = pf_pool.tile([128, 1024], bf16)
                            nc.vector.tensor_scalar(
                                pf[:].bitcast(i16), sp[:], SCHA, SCHB,
                                Alu.mult, Alu.add)
                            pt_ap, vx_ap = pf[:], vb[:]
                        else:
                            pb = pb_pool.tile([128, 1024], bf16)
                            if probe == "smallexp":
                                nc.scalar.activation(
                                    pb[:, 0:256], sp[:, 0:256],
                                    mybir.ActivationFunctionType.Exp,
                                    scale=SCALE)
                            else:
                                nc.scalar.activation(
                                    pb[:], sp[:],
                                    mybir.ActivationFunctionType.Exp,
                                    scale=SCALE)
                            pt_ap, vx_ap = pb[:], vb[:]
                        if c == 1 and pending is not None:
                            emit_epilogue(*pending)
                            pending = None
                        if prev is not None:
                            emit_av(*prev)
                        prev = (c, pt_ap, vx_ap)
                    emit_av(*prev)
                    pending = (h, qgp, av)
            if pending is not None:
                emit_epilogue(*pending)

        if hw_loop_reps > 1:
            with tc.For_i(0, hw_loop_reps):
                emit_body()
        else:
            for rep in range(reps):
                emit_body()

    nc.compile()
    return nc


def _build_nc3(d_dve=2, schc=0.045, epicopy="dve", probe="", reps=1,
               hw_loop_reps=1, hw_unroll=1):
    """v3: v1 skeleton (512-q groups, sps depth 3) + bf16 AV everywhere +
    ACT/DVE exp split (DVE = int16 Schraudolph -> bf16 bits) + host-packed V
    + batched epilogue reciprocal."""
    import concourse.tile as tile
    from concourse import bacc, mybir
    from concourse.masks import make_identity
    from contextlib import ExitStack

    f32 = mybir.dt.float32
    bf16 = mybir.dt.bfloat16
    i16 = mybir.dt.int16
    Alu = mybir.AluOpType

    # group gi (2 chunks, FD=1024) -> DVE when gi in dve_gset; gi 0,1 stay
    # ACT. d_dve=7: no whole-DVE groups; instead EVERY group is split 50/50,
    # ACT on PSUM bank 0 / DVE on bank 1, concurrently (bank-aligned so the
    # engines never contend for a PSUM bank port).
    assert 0 <= d_dve <= 7
    split_mode = d_dve == 7
    if d_dve == 0 or split_mode:
        dve_gset = set()
    elif d_dve == 1:
        dve_gset = {4}
    else:
        dve_gset = {2 + round(i * 5 / (d_dve - 1)) for i in range(d_dve)}
    SCHA = float(SCALE * LOG2E * (1 << 7))
    SCHB = float(127.0 * (1 << 7) - schc * (1 << 7))

    nc = bacc.Bacc("TRN2", target_bir_lowering=False, debug=False)

    qt_d = nc.dram_tensor("qt", [H_PER, D, S], bf16, kind="ExternalInput").ap()
    kt_d = nc.dram_tensor("kt", [H_PER, D, S], bf16, kind="ExternalInput").ap()
    vb_d = nc.dram_tensor("vb", [H_PER, 128, NT * 80], bf16,
                          kind="ExternalInput").ap()
    o_d = nc.dram_tensor("out", [H_PER, S, D], f32, kind="ExternalOutput").ap()

    with tile.TileContext(nc) as tc, ExitStack() as ctx:
        qt_pool = ctx.enter_context(tc.tile_pool(name="qt", bufs=2))
        kt_pool = ctx.enter_context(tc.tile_pool(name="kt", bufs=2))
        vb_pool = ctx.enter_context(tc.tile_pool(name="vb", bufs=2))
        pb_pool = ctx.enter_context(tc.tile_pool(name="pb", bufs=6))
        if dve_gset:
            pf_pool = ctx.enter_context(tc.tile_pool(name="pf", bufs=6))
        o_pool = ctx.enter_context(tc.tile_pool(name="op", bufs=2))
        r_pool = ctx.enter_context(tc.tile_pool(name="rp", bufs=3))
        res_pool = ctx.enter_context(tc.tile_pool(name="resp", bufs=3))
        id_pool = ctx.enter_context(tc.tile_pool(name="idp", bufs=1))
        sps_pool = ctx.enter_context(
            tc.tile_pool(name="sps", bufs=3, space="PSUM"))
        av_pool = ctx.enter_context(
            tc.tile_pool(name="av", bufs=1, space="PSUM"))
        tps_pool = ctx.enter_context(
            tc.tile_pool(name="tps", bufs=1, space="PSUM"))

        ident = id_pool.tile([128, 128], f32)
        make_identity(nc, ident[:])

        def emit_load(h):
            qt = qt_pool.tile([D, S], bf16)
            kt = kt_pool.tile([D, S], bf16)
            nc.gpsimd.dma_start(kt[:, 0:512], kt_d[h, :, 0:512])
            nc.gpsimd.dma_start(qt[:, 0:512], qt_d[h, :, 0:512])
            nc.gpsimd.dma_start(kt[:, 512:S], kt_d[h, :, 512:S])
            nc.gpsimd.dma_start(qt[:, 512:S], qt_d[h, :, 512:S])
            vb = vb_pool.tile([128, NT * 80], bf16)
            nc.gpsimd.dma_start(vb[:], vb_d[h])
            return qt, kt, vb

        def emit_epilogue(h, qg, av):
            sb = o_pool.tile([80, 512], f32)
            if epicopy == "act":
                nc.scalar.copy(sb[:], av[:])
            else:
                nc.vector.tensor_copy(sb[:], av[:])
            tp = tps_pool.tile([128, 4 * 80], f32)
            tp3 = tp[:].rearrange("p (t e) -> p t e", e=80)
            for t in range(4):
                nc.tensor.transpose(
                    tp[:, t * 80:(t + 1) * 80],
                    sb[:, t * 128:(t + 1) * 128],
                    ident[0:80, 0:80],
                )
            rec = r_pool.tile([128, 4], f32)
            nc.vector.reciprocal(
                rec[:].rearrange("p (t e) -> p t e", e=1), tp3[:, :, 64:65])
            res = res_pool.tile([128, 4 * 64], f32)
            res3 = res[:].rearrange("p (t d) -> p t d", d=64)
            for t in range(4):
                nc.vector.tensor_scalar_mul(
                    res3[:, t, :], tp3[:, t, 0:64], rec[:, t:t + 1])
            nc.sync.dma_start(
                o_d[h, qg * 512:(qg + 1) * 512, :].rearrange(
                    "(t p) d -> p t d", p=128),
                res3,
            )

        def emit_body():
            pending = None
            for h in range(H_PER):
                qt, kt, vb = emit_load(h)
                for qg in range(QG):
                    av = av_pool.tile([80, 512], f32)

                    def emit_av(gi, pt_ap):
                        for i in range(2):
                            c = 2 * gi + i
                            if "noav" in probe and c > 0:
                                continue
                            nc.tensor.matmul(
                                av[:],
                                lhsT=vb[:, c * 80:(c + 1) * 80],
                                rhs=pt_ap[:, i * 512:(i + 1) * 512],
                                start=(c == 0),
                                stop=(c == NT - 1) or "noav" in probe,
                            )

                    avq = []
                    for gi in range(8):
                        sp = sps_pool.tile([128, 1024], f32)
                        qkw = 128 if "smallqk" in probe else 512
                        for i in range(2):
                            nc.tensor.matmul(
                                sp[:, i * 512:i * 512 + qkw],
                                lhsT=kt[:, (2 * gi + i) * 128:
                                        (2 * gi + i + 1) * 128],
                                rhs=qt[:, qg * 512:qg * 512 + qkw],
                                start=True,
                                stop=True,
                            )
                        if split_mode:
                            pb = pb_pool.tile([128, 1024], bf16)
                            nc.scalar.activation(
                                pb[:, 0:512], sp[:, 0:512],
                                mybir.ActivationFunctionType.Exp,
                                scale=SCALE)
                            nc.vector.tensor_scalar(
                                pb[:, 512:1024].bitcast(i16),
                                sp[:, 512:1024], SCHA, SCHB,
                                Alu.mult, Alu.add)
                            pt_ap = pb[:]
                        elif gi in dve_gset:
                            pf = pf_pool.tile([128, 1024], bf16)
                            nc.vector.tensor_scalar(
                                pf[:].bitcast(i16), sp[:], SCHA, SCHB,
                                Alu.mult, Alu.add)
                            pt_ap = pf[:]
                        else:
                            pb = pb_pool.tile([128, 1024], bf16)
                            if "smallexp" in probe:
                                nc.scalar.activation(
                                    pb[:, 0:256], sp[:, 0:256],
                                    mybir.ActivationFunctionType.Exp,
                                    scale=SCALE)
                            else:
                                nc.scalar.activation(
                                    pb[:], sp[:],
                                    mybir.ActivationFunctionType.Exp,
                                    scale=SCALE)
                            pt_ap = pb[:]
                        if gi == 1 and pending is not None:
                            emit_epilogue(*pending)
                            pending = None
                        # AV lags 2 groups so the PE stream never stalls on
                        # an exp semaphore ahead of the next QK matmuls
                        if len(avq) == 2:
                            emit_av(*avq.pop(0))
                        avq.append((gi, pt_ap))
                    for item in avq:
                        emit_av(*item)
                    avq.clear()
                    pending = (h, qg, av)
            if pending is not None:
                emit_epilogue(*pending)

        if hw_loop_reps > 1:
            with tc.For_i(0, hw_loop_reps):
                for _ in range(hw_unroll):
                    emit_body()
        else:
            for rep in range(reps):
                emit_body()

    nc.compile()
    return nc


def _build_nc9(d_dve=2, schc=0.045, epicopy="dve", avp=0, probe="", reps=1,
               hw_loop_reps=1, hw_unroll=1):
    """v9: v3 EXACTLY, except Q^T/K^T are row-duplicated on the host to
    [128, S] ([X; X]) and each group's two QK chunk-matmuls are emitted at
    tile_position (0,0) / (64,0) using the two row copies. Consecutive QK
    MMs then differ in row group, so their LDWEIGHTS pull ahead and the two
    N=512 streams overlap (bench_pair.py: 247 ns/pair vs 424 ns each at the
    same position). They write different PSUM banks (sp col halves), so no
    drain race. AV, exp split, epilogue, PSUM budget (sps 3x2 + av + tps =
    8) are untouched v3.
    """
    import concourse.tile as tile
    from concourse import bacc, mybir
    from concourse.masks import make_identity
    from contextlib import ExitStack

    f32 = mybir.dt.float32
    bf16 = mybir.dt.bfloat16
    i16 = mybir.dt.int16
    Alu = mybir.AluOpType

    assert 0 <= d_dve <= 7
    split_mode = d_dve == 7
    if d_dve == 0 or split_mode:
        dve_gset = set()
    elif d_dve == 1:
        dve_gset = {4}
    else:
        dve_gset = {2 + round(i * 5 / (d_dve - 1)) for i in range(d_dve)}
    SCHA = float(SCALE * LOG2E * (1 << 7))
    SCHB = float(127.0 * (1 << 7) - schc * (1 << 7))

    nc = bacc.Bacc("TRN2", target_bir_lowering=False, debug=False)

    qt_d = nc.dram_tensor("qt", [H_PER, 128, S], bf16,
                          kind="ExternalInput").ap()
    kt_d = nc.dram_tensor("kt", [H_PER, 128, S], bf16,
                          kind="ExternalInput").ap()
    vb_d = nc.dram_tensor("vb", [H_PER, 128, NT * 80], bf16,
                          kind="ExternalInput").ap()
    o_d = nc.dram_tensor("out", [H_PER, S, D], f32, kind="ExternalOutput").ap()

    with tile.TileContext(nc) as tc, ExitStack() as ctx:
        qt_pool = ctx.enter_context(tc.tile_pool(name="qt", bufs=2))
        kt_pool = ctx.enter_context(tc.tile_pool(name="kt", bufs=2))
        vb_pool = ctx.enter_context(tc.tile_pool(name="vb", bufs=2))
        pb_pool = ctx.enter_context(tc.tile_pool(name="pb", bufs=6))
        if dve_gset:
            pf_pool = ctx.enter_context(tc.tile_pool(name="pf", bufs=6))
        o_pool = ctx.enter_context(tc.tile_pool(name="op", bufs=2))
        r_pool = ctx.enter_context(tc.tile_pool(name="rp", bufs=3))
        res_pool = ctx.enter_context(tc.tile_pool(name="resp", bufs=3))
        id_pool = ctx.enter_context(tc.tile_pool(name="idp", bufs=1))
        # avp mode needs a second accumulator bank; pay for it with sps=2
        sps_pool = ctx.enter_context(
            tc.tile_pool(name="sps", bufs=(2 if avp else 3), space="PSUM"))
        av_pool = ctx.enter_context(
            tc.tile_pool(name="av", bufs=1, space="PSUM"))
        tps_pool = ctx.enter_context(
            tc.tile_pool(name="tps", bufs=1, space="PSUM"))

        ident = id_pool.tile([128, 128], f32)
        make_identity(nc, ident[:])
        if avp:
            so_pool = ctx.enter_context(tc.tile_pool(name="sop", bufs=2))
            idb_pool = ctx.enter_context(tc.tile_pool(name="idb", bufs=1))
            identb = idb_pool.tile([128, 128], bf16)
            nc.vector.tensor_copy(identb[:], ident[:])

        def emit_load(h):
            qt = qt_pool.tile([128, S], bf16)
            kt = kt_pool.tile([128, S], bf16)
            nc.gpsimd.dma_start(kt[:, 0:512], kt_d[h, :, 0:512])
            nc.gpsimd.dma_start(qt[:, 0:512], qt_d[h, :, 0:512])
            nc.gpsimd.dma_start(kt[:, 512:S], kt_d[h, :, 512:S])
            nc.gpsimd.dma_start(qt[:, 512:S], qt_d[h, :, 512:S])
            vb = vb_pool.tile([128, NT * 80], bf16)
            nc.gpsimd.dma_start(vb[:], vb_d[h])
            return qt, kt, vb

        def emit_epilogue(h, qg, av):
            sb = o_pool.tile([80, 512], f32)
            if epicopy == "act":
                nc.scalar.copy(sb[:], av[:])
            else:
                nc.vector.tensor_copy(sb[:], av[:])
            tp = tps_pool.tile([128, 4 * 80], f32)
            tp3 = tp[:].rearrange("p (t e) -> p t e", e=80)
            for t in range(4):
                nc.tensor.transpose(
                    tp[:, t * 80:(t + 1) * 80],
                    sb[:, t * 128:(t + 1) * 128],
                    ident[0:80, 0:80],
                )
            rec = r_pool.tile([128, 4], f32)
            nc.vector.reciprocal(
                rec[:].rearrange("p (t e) -> p t e", e=1), tp3[:, :, 64:65])
            res = res_pool.tile([128, 4 * 64], f32)
            res3 = res[:].rearrange("p (t d) -> p t d", d=64)
            for t in range(4):
                nc.vector.tensor_scalar_mul(
                    res3[:, t, :], tp3[:, t, 0:64], rec[:, t:t + 1])
            nc.sync.dma_start(
                o_d[h, qg * 512:(qg + 1) * 512, :].rearrange(
                    "(t p) d -> p t d", p=128),
                res3,
            )

        def emit_body():
            pending = None
            for h in range(H_PER):
                qt, kt, vb = emit_load(h)
                for qg in range(QG):
                    av = av_pool.tile([80, 512], f32, name="ava")
                    avo = (av_pool.tile([80, 512], f32, name="avo")
                           if avp else None)

                    def emit_av(gi, pt_ap):
                        for i in range(2):
                            c = 2 * gi + i
                            if "noav" in probe and c > 0:
                                continue
                            if avp:
                                # lo half -> av @(0,0) || hi half -> avo
                                # @(64,0): concurrent pair into different
                                # banks; chain-internal order is fenced by
                                # the same-row-group cell conflict.
                                nc.tensor.matmul(
                                    av[:],
                                    lhsT=vb[0:64, c * 80:(c + 1) * 80],
                                    rhs=pt_ap[0:64,
                                              i * 512:(i + 1) * 512],
                                    start=(c == 0), stop=False,
                                    tile_position=(0, 0),
                                )
                                nc.tensor.matmul(
                                    avo[:],
                                    lhsT=vb[64:128, c * 80:(c + 1) * 80],
                                    rhs=pt_ap[64:128,
                                              i * 512:(i + 1) * 512],
                                    start=(c == 0),
                                    stop=(c == NT - 1) or "noav" in probe,
                                    tile_position=(64, 0),
                                )
                            else:
                                nc.tensor.matmul(
                                    av[:],
                                    lhsT=vb[:, c * 80:(c + 1) * 80],
                                    rhs=pt_ap[:, i * 512:(i + 1) * 512],
                                    start=(c == 0),
                                    stop=(c == NT - 1) or "noav" in probe,
                                )

                    def emit_av_merge():
                        # avo (hi-half sums) -> bf16 SBUF -> one identity
                        # matmul accumulates onto av, closing av's group.
                        sbo = so_pool.tile([80, 512], bf16)
                        nc.vector.tensor_copy(sbo[:], avo[:])
                        nc.tensor.matmul(
                            av[:],
                            lhsT=identb[0:80, 0:80],
                            rhs=sbo[:],
                            start=False, stop=True,
                        )

                    avq = []
                    for gi in range(8):
                        sp = sps_pool.tile([128, 1024], f32)
                        qkw = 128 if "smallqk" in probe else 512
                        for i in range(2):
                            nc.tensor.matmul(
                                sp[:, i * 512:i * 512 + qkw],
                                lhsT=kt[i * 64:(i + 1) * 64,
                                        (2 * gi + i) * 128:
                                        (2 * gi + i + 1) * 128],
                                rhs=qt[i * 64:(i + 1) * 64,
                                       qg * 512:qg * 512 + qkw],
                                start=True,
                                stop=True,
                                tile_position=(i * 64, 0),
                            )
                        if gi in dve_gset:
                            pf = pf_pool.tile([128, 1024], bf16)
                            nc.vector.tensor_scalar(
                                pf[:].bitcast(i16), sp[:], SCHA, SCHB,
                                Alu.mult, Alu.add)
                            pt_ap = pf[:]
                        else:
                            pb = pb_pool.tile([128, 1024], bf16)
                            if "smallexp" in probe:
                                nc.scalar.activation(
                                    pb[:, 0:256], sp[:, 0:256],
                                    mybir.ActivationFunctionType.Exp,
                                    scale=SCALE)
                            else:
                                nc.scalar.activation(
                                    pb[:], sp[:],
                                    mybir.ActivationFunctionType.Exp,
                                    scale=SCALE)
                            pt_ap = pb[:]
                        if gi == 1 and pending is not None:
                            emit_epilogue(*pending)
                            pending = None
                        if len(avq) == 2:
                            emit_av(*avq.pop(0))
                        avq.append((gi, pt_ap))
                    for item in avq:
                        emit_av(*item)
                    avq.clear()
                    if avp:
                        emit_av_merge()
                    pending = (h, qg, av)
            if pending is not None:
                emit_epilogue(*pending)

        if hw_loop_reps > 1:
            with tc.For_i(0, hw_loop_reps):
                for _ in range(hw_unroll):
                    emit_body()
        else:
            for rep in range(reps):
                emit_body()

    nc.compile()
    return nc


def _build_nc4(d_dve=6, schc=0.045, epicopy="dve", probe="", reps=1,
               hw_loop_reps=1, hw_unroll=1):
    """v4: v3 + row-tiled head-paired QK. Heads are processed in pairs; the
    two heads' K-chunk weights occupy PE rows 0-63 / 64-127 (tile_position
    (0,0) / (64,0)) and stream concurrently, nearly halving QK PE time.
    Score tile [128, 1024] = chunk c for head A (cols 0:512) + head B
    (cols 512:1024); one FD=1024 exp call covers both."""
    import concourse.tile as tile
    from concourse import bacc, mybir
    from concourse.masks import make_identity
    from contextlib import ExitStack

    f32 = mybir.dt.float32
    bf16 = mybir.dt.bfloat16
    i16 = mybir.dt.int16
    Alu = mybir.AluOpType

    dve_set = _dve_chunks(d_dve)
    SCHA = float(SCALE * LOG2E * (1 << 7))
    SCHB = float(127.0 * (1 << 7) - schc * (1 << 7))

    nc = bacc.Bacc("TRN2", target_bir_lowering=False, debug=False)

    # head-pair layout: qt2[hp] = [128, S] with head 2hp on partitions 0-63,
    # head 2hp+1 on 64-127 (plain reshape of the [4, 64, S] layout)
    qt_d = nc.dram_tensor("qt", [H_PER // 2, 128, S], bf16,
                          kind="ExternalInput").ap()
    kt_d = nc.dram_tensor("kt", [H_PER // 2, 128, S], bf16,
                          kind="ExternalInput").ap()
    vb_d = nc.dram_tensor("vb", [H_PER, 128, NT * 80], bf16,
                          kind="ExternalInput").ap()
    o_d = nc.dram_tensor("out", [H_PER, S, D], f32, kind="ExternalOutput").ap()

    with tile.TileContext(nc) as tc, ExitStack() as ctx:
        qt_pool = ctx.enter_context(tc.tile_pool(name="qt", bufs=2))
        kt_pool = ctx.enter_context(tc.tile_pool(name="kt", bufs=2))
        vb_pool = ctx.enter_context(tc.tile_pool(name="vb", bufs=2))
        pb_pool = ctx.enter_context(tc.tile_pool(name="pb", bufs=4))
        if dve_set:
            pf_pool = ctx.enter_context(tc.tile_pool(name="pf", bufs=4))
        o_pool = ctx.enter_context(tc.tile_pool(name="op", bufs=2))
        r_pool = ctx.enter_context(tc.tile_pool(name="rp", bufs=2))
        res_pool = ctx.enter_context(tc.tile_pool(name="resp", bufs=2))
        id_pool = ctx.enter_context(tc.tile_pool(name="idp", bufs=1))
        sps_pool = ctx.enter_context(
            tc.tile_pool(name="sps", bufs=2, space="PSUM"))
        av_pool = ctx.enter_context(
            tc.tile_pool(name="av", bufs=1, space="PSUM"))
        tps_pool = ctx.enter_context(
            tc.tile_pool(name="tps", bufs=2, space="PSUM"))

        ident = id_pool.tile([128, 128], f32)
        make_identity(nc, ident[:])

        def emit_load(hp):
            qt = qt_pool.tile([128, S], bf16)
            kt = kt_pool.tile([128, S], bf16)
            nc.sync.dma_start(kt[:, 0:512], kt_d[hp, :, 0:512])
            nc.sync.dma_start(qt[:, 0:512], qt_d[hp, :, 0:512])
            nc.sync.dma_start(kt[:, 512:S], kt_d[hp, :, 512:S])
            nc.sync.dma_start(qt[:, 512:S], qt_d[hp, :, 512:S])
            vba = vb_pool.tile([128, NT * 80], bf16, name="vba")
            vbb = vb_pool.tile([128, NT * 80], bf16, name="vbb")
            nc.sync.dma_start(vba[:], vb_d[2 * hp])
            nc.sync.dma_start(vbb[:], vb_d[2 * hp + 1])
            return qt, kt, vba, vbb

        def emit_epilogue(h, qg, av):
            sb = o_pool.tile([80, 512], f32)
            if epicopy == "act":
                nc.scalar.copy(sb[:], av[:])
            else:
                nc.vector.tensor_copy(sb[:], av[:])
            tp = tps_pool.tile([128, 4 * 80], f32)
            tp3 = tp[:].rearrange("p (t e) -> p t e", e=80)
            for t in range(4):
                nc.tensor.transpose(
                    tp[:, t * 80:(t + 1) * 80],
                    sb[:, t * 128:(t + 1) * 128],
                    ident[0:80, 0:80],
                )
            rec = r_pool.tile([128, 4], f32)
            nc.vector.reciprocal(
                rec[:].rearrange("p (t e) -> p t e", e=1), tp3[:, :, 64:65])
            res = res_pool.tile([128, 4 * 64], f32)
            res3 = res[:].rearrange("p (t d) -> p t d", d=64)
            for t in range(4):
                nc.vector.tensor_scalar_mul(
                    res3[:, t, :], tp3[:, t, 0:64], rec[:, t:t + 1])
            nc.sync.dma_start(
                o_d[h, qg * 512:(qg + 1) * 512, :].rearrange(
                    "(t p) d -> p t d", p=128),
                res3,
            )

        def emit_body():
            pending = []
            for hp in range(H_PER // 2):
                qt, kt, vba, vbb = emit_load(hp)
                for qg in range(QG):
                    ava = av_pool.tile([80, 512], f32, name="ava")
                    avb = av_pool.tile([80, 512], f32, name="avb")

                    def emit_av(c, pt_ap):
                        if "noav" in probe and c > 0:
                            return
                        stop = (c == NT - 1) or "noav" in probe
                        nc.tensor.matmul(
                            ava[:],
                            lhsT=vba[:, c * 80:(c + 1) * 80],
                            rhs=pt_ap[:, 0:512],
                            start=(c == 0), stop=stop,
                        )
                        nc.tensor.matmul(
                            avb[:],
                            lhsT=vbb[:, c * 80:(c + 1) * 80],
                            rhs=pt_ap[:, 512:1024],
                            start=(c == 0), stop=stop,
                        )

                    avq = []
                    for c in range(NT):
                        sp = sps_pool.tile([128, 1024], f32)
                        qkw = 128 if "smallqk" in probe else 512
                        for half in range(2):
                            nc.tensor.matmul(
                                sp[:, half * 512:half * 512 + qkw],
                                lhsT=kt[half * 64:(half + 1) * 64,
                                        c * 128:(c + 1) * 128],
                                rhs=qt[half * 64:(half + 1) * 64,
                                       qg * 512:qg * 512 + qkw],
                                start=True,
                                stop=True,
                                tile_position=(half * 64, 0),
                            )
                        if c in dve_set:
                            pf = pf_pool.tile([128, 1024], bf16)
                            nc.vector.tensor_scalar(
                                pf[:].bitcast(i16), sp[:], SCHA, SCHB,
                                Alu.mult, Alu.add)
                            pt_ap = pf[:]
                        else:
                            pb = pb_pool.tile([128, 1024], bf16)
                            if "smallexp" in probe:
                                nc.scalar.activation(
                                    pb[:, 0:256], sp[:, 0:256],
                                    mybir.ActivationFunctionType.Exp,
                                    scale=SCALE)
                            else:
                                nc.scalar.activation(
                                    pb[:], sp[:],
                                    mybir.ActivationFunctionType.Exp,
                                    scale=SCALE)
                            pt_ap = pb[:]
                        if c in (1, 2) and pending:
                            emit_epilogue(*pending.pop(0))
                        if len(avq) == 2:
                            emit_av(*avq.pop(0))
                        avq.append((c, pt_ap))
                    for item in avq:
                        emit_av(*item)
                    avq.clear()
                    pending = [(2 * hp, qg, ava), (2 * hp + 1, qg, avb)]
            for item in pending:
                emit_epilogue(*item)

        if hw_loop_reps > 1:
            with tc.For_i(0, hw_loop_reps):
                for _ in range(hw_unroll):
                    emit_body()
        else:
            for rep in range(reps):
                emit_body()

    nc.compile()
    return nc


def _build_nc5(d_dve=13, schc=0.045, epimul="act", sps_bufs=2, probe="", reps=1,
               hw_loop_reps=1, hw_unroll=1):
    """v5: transposed-AV ("AVT") attention.

    Per head pair (2 heads stacked on partitions 0-63 / 64-127):
      - QK^T per (qg, kc): two matmuls (tile_position (0,0)/(64,0)) into two
        1-bank fp32 PSUM score tiles spA/spB [128 k, 512 q].
      - exp: 32 calls per (pair, qg) of FD=512, split ACT (exp activation)
        vs DVE (Schraudolph int16 bf16-bits) by slot -> p tiles [128, 512]
        bf16 in SBUF (k on partitions, q on free).
      - AVT: p tile is the STATIONARY operand ([128 k, 128 q-block] weights,
        FWL-eligible), V chunk [128 k, 68] streams (cols 0:64 V, 64 ones,
        65:68 zero) -> out [128 q, 68] f32 accumulated over all 16 k-chunks
        in PSUM; col 64 accumulates the softmax denominator. No PE transpose
        epilogue at all; out is already [q, d].
      - 4 q-block accumulator chains share one PSUM bank per head; since
        start=True clears has_written for the WHOLE bank, the bank is
        pre-zeroed by one [1x128]@[1x272] zero matmul and all chains run
        start=False.
      - Epilogue per (head, qg): reciprocal of col 64 + 4 per-partition
        scalar muls (ACT or DVE) -> [128, 4*64] SBUF -> DMA.
    PSUM: spA/spB (sps_bufs each) + avA/avB (2 each) = 2*sps_bufs + 4 banks.
    """
    import concourse.tile as tile
    from concourse import bacc, mybir
    from contextlib import ExitStack

    f32 = mybir.dt.float32
    bf16 = mybir.dt.bfloat16
    i16 = mybir.dt.int16
    Alu = mybir.AluOpType

    # slot = 2*c + head (0..31); slots 0,1 (c=0) stay ACT.
    assert 0 <= d_dve <= 30
    dve_set = set()
    if d_dve == 1:
        dve_set = {16}
    elif d_dve > 1:
        dve_set = {2 + round(i * 29 / (d_dve - 1)) for i in range(d_dve)}
        # fill any rounding collisions
        j = 2
        while len(dve_set) < d_dve:
            if j not in dve_set:
                dve_set.add(j)
            j += 1
    SCHA = float(SCALE * LOG2E * (1 << 7))
    SCHB = float(127.0 * (1 << 7) - schc * (1 << 7))

    nc = bacc.Bacc("TRN2", target_bir_lowering=False, debug=False)

    qt_d = nc.dram_tensor("qt", [H_PER // 2, 128, S], bf16,
                          kind="ExternalInput").ap()
    kt_d = nc.dram_tensor("kt", [H_PER // 2, 128, S], bf16,
                          kind="ExternalInput").ap()
    vb_d = nc.dram_tensor("vb", [H_PER, 128, NT * 68], bf16,
                          kind="ExternalInput").ap()
    o_d = nc.dram_tensor("out", [H_PER, S, D], f32, kind="ExternalOutput").ap()

    with tile.TileContext(nc) as tc, ExitStack() as ctx:
        qt_pool = ctx.enter_context(tc.tile_pool(name="qt", bufs=2))
        kt_pool = ctx.enter_context(tc.tile_pool(name="kt", bufs=2))
        vb_pool = ctx.enter_context(tc.tile_pool(name="vb", bufs=2))
        p_pool = ctx.enter_context(tc.tile_pool(name="pp", bufs=4))
        z_pool = ctx.enter_context(tc.tile_pool(name="zp", bufs=1))
        r_pool = ctx.enter_context(tc.tile_pool(name="rp", bufs=3))
        res_pool = ctx.enter_context(tc.tile_pool(name="resp", bufs=3))
        sps_pool = ctx.enter_context(
            tc.tile_pool(name="sps", bufs=sps_bufs, space="PSUM"))
        av_pool = ctx.enter_context(
            tc.tile_pool(name="av", bufs=2, space="PSUM"))

        zt = z_pool.tile([1, 512], bf16)
        nc.vector.memset(zt[:], 0.0)

        def emit_load(hp):
            qt = qt_pool.tile([128, S], bf16)
            kt = kt_pool.tile([128, S], bf16)
            nc.sync.dma_start(kt[:, 0:512], kt_d[hp, :, 0:512])
            nc.sync.dma_start(qt[:, 0:512], qt_d[hp, :, 0:512])
            vba = vb_pool.tile([128, NT * 68], bf16, name="vba")
            vbb = vb_pool.tile([128, NT * 68], bf16, name="vbb")
            nc.sync.dma_start(vba[:], vb_d[2 * hp])
            nc.sync.dma_start(vbb[:], vb_d[2 * hp + 1])
            nc.sync.dma_start(kt[:, 512:S], kt_d[hp, :, 512:S])
            nc.sync.dma_start(qt[:, 512:S], qt_d[hp, :, 512:S])
            return qt, kt, vba, vbb

        def emit_epilogue(h, qg, av):
            av3 = av[:].rearrange("p (t e) -> p t e", e=68)
            rec = r_pool.tile([128, 4], f32)
            nc.vector.reciprocal(
                rec[:].rearrange("p (t e) -> p t e", e=1), av3[:, :, 64:65])
            res = res_pool.tile([128, 4 * 64], f32)
            res3 = res[:].rearrange("p (t d) -> p t d", d=64)
            for t in range(4):
                on_act = epimul == "act" or (epimul == "mix" and t % 2 == 0)
                if on_act:
                    nc.scalar.mul(res3[:, t, :], av3[:, t, 0:64],
                                  rec[:, t:t + 1])
                else:
                    nc.vector.tensor_scalar_mul(
                        res3[:, t, :], av3[:, t, 0:64], rec[:, t:t + 1])
            nc.sync.dma_start(
                o_d[h, qg * 512:(qg + 1) * 512, :].rearrange(
                    "(t p) d -> p t d", p=128),
                res3,
            )

        def emit_body():
            pending = []
            for hp in range(H_PER // 2):
                qt, kt, vba, vbb = emit_load(hp)
                for qg in range(QG):
                    avA = av_pool.tile([128, 4 * 68], f32, name="avA")
                    avB = av_pool.tile([128, 4 * 68], f32, name="avB")
                    # pre-zero the accumulator banks (sets has_written so the
                    # start=False chains below accumulate onto 0)
                    for av in (avA, avB):
                        nc.tensor.matmul(
                            av[:], lhsT=zt[0:1, 0:128], rhs=zt[0:1, 0:272],
                            start=True, stop=False, skip_group_check=True)

                    def emit_avt(c, p_ap, av, vb):
                        for qb in range(4):
                            if "noav" in probe and c > 0:
                                continue
                            nc.tensor.matmul(
                                av[:, qb * 68:qb * 68 + 68],
                                lhsT=p_ap[:, qb * 128:(qb + 1) * 128],
                                rhs=vb[:, c * 68:(c + 1) * 68],
                                start=False,
                                stop=(c == NT - 1) or "noav" in probe,
                                skip_group_check=True,
                            )

                    def emit_exp(c, head, sp):
                        p = p_pool.tile([128, 512], bf16,
                                        name=f"p{'ab'[head]}")
                        w = 128 if "smallexp" in probe else 512
                        if (2 * c + head) in dve_set:
                            nc.vector.tensor_scalar(
                                p[:, 0:w].bitcast(i16), sp[:, 0:w],
                                SCHA, SCHB, Alu.mult, Alu.add)
                        else:
                            nc.scalar.activation(
                                p[:, 0:w], sp[:, 0:w],
                                mybir.ActivationFunctionType.Exp,
                                scale=SCALE)
                        return p

                    avq = []
                    for c in range(NT):
                        qkw = 128 if "smallqk" in probe else 512
                        item = avq.pop(0) if len(avq) == 2 else None
                        if item is not None:
                            emit_avt(item[0], item[1], avA, vba)
                        spA = sps_pool.tile([128, 512], f32, name="spA")
                        nc.tensor.matmul(
                            spA[:, 0:qkw],
                            lhsT=kt[0:64, c * 128:(c + 1) * 128],
                            rhs=qt[0:64, qg * 512:qg * 512 + qkw],
                            start=True, stop=True, tile_position=(0, 0))
                        if item is not None:
                            emit_avt(item[0], item[2], avB, vbb)
                        spB = sps_pool.tile([128, 512], f32, name="spB")
                        nc.tensor.matmul(
                            spB[:, 0:qkw],
                            lhsT=kt[64:128, c * 128:(c + 1) * 128],
                            rhs=qt[64:128, qg * 512:qg * 512 + qkw],
                            start=True, stop=True, tile_position=(64, 0))
                        pA = emit_exp(c, 0, spA)
                        pB = emit_exp(c, 1, spB)
                        if c in (1, 2) and pending:
                            emit_epilogue(*pending.pop(0))
                        avq.append((c, pA, pB))
                    for item in avq:
                        emit_avt(item[0], item[1], avA, vba)
                        emit_avt(item[0], item[2], avB, vbb)
                    avq.clear()
                    pending = [(2 * hp, qg, avA), (2 * hp + 1, qg, avB)]
            for item in pending:
                emit_epilogue(*item)

        if hw_loop_reps > 1:
            with tc.For_i(0, hw_loop_reps):
                for _ in range(hw_unroll):
                    emit_body()
        else:
            for rep in range(reps):
                emit_body()

    nc.compile()
    return nc


def _build_nc6(d_dve=8, schc=0.045, epimul="mix", epicopy="dve", probe="",
               reps=1, hw_loop_reps=1, hw_unroll=1):
    """v6: v3 flow with (a) head-pair-stacked QK (two row-tiled matmuls at
    tile_position (0,0)/(64,0) share one [128, 1024] score tile = [A | B]),
    (b) DMA-transpose epilogue (no PE transposes, no tps PSUM bank: out^T
    [80,512] -> bf16 SBUF -> 4 transpose-DMAs -> [128, 4*80] -> recip + 4
    per-partition muls, bf16 res, host upcasts), (c) exp split ACT/DVE by
    k-chunk (FD=1024 calls), epilogue ops mixed across ACT/DVE.
    PSUM: sps 3x2 banks + avA + avB = 8.
    """
    import concourse.tile as tile
    from concourse import bacc, mybir
    from contextlib import ExitStack

    f32 = mybir.dt.float32
    bf16 = mybir.dt.bfloat16
    i16 = mybir.dt.int16
    Alu = mybir.AluOpType

    assert 0 <= d_dve <= 14
    if d_dve == 0:
        dve_set = set()
    elif d_dve == 1:
        dve_set = {8}
    else:
        dve_set = {2 + round(i * 13 / (d_dve - 1)) for i in range(d_dve)}
        j = 2
        while len(dve_set) < d_dve:
            if j not in dve_set:
                dve_set.add(j)
            j += 1
    SCHA = float(SCALE * LOG2E * (1 << 7))
    SCHB = float(127.0 * (1 << 7) - schc * (1 << 7))

    nc = bacc.Bacc("TRN2", target_bir_lowering=False, debug=False)

    qt_d = nc.dram_tensor("qt", [H_PER // 2, 128, S], bf16,
                          kind="ExternalInput").ap()
    kt_d = nc.dram_tensor("kt", [H_PER // 2, 128, S], bf16,
                          kind="ExternalInput").ap()
    vb_d = nc.dram_tensor("vb", [H_PER, 128, NT * 80], bf16,
                          kind="ExternalInput").ap()
    o_d = nc.dram_tensor("out", [H_PER, S, D], bf16, kind="ExternalOutput").ap()

    with tile.TileContext(nc) as tc, ExitStack() as ctx:
        qt_pool = ctx.enter_context(tc.tile_pool(name="qt", bufs=2))
        kt_pool = ctx.enter_context(tc.tile_pool(name="kt", bufs=2))
        vb_pool = ctx.enter_context(tc.tile_pool(name="vb", bufs=2))
        pb_pool = ctx.enter_context(tc.tile_pool(name="pb", bufs=6))
        o_pool = ctx.enter_context(tc.tile_pool(name="op", bufs=2))
        x_pool = ctx.enter_context(tc.tile_pool(name="xp", bufs=2))
        r_pool = ctx.enter_context(tc.tile_pool(name="rp", bufs=3))
        res_pool = ctx.enter_context(tc.tile_pool(name="resp", bufs=3))
        sps_pool = ctx.enter_context(
            tc.tile_pool(name="sps", bufs=3, space="PSUM"))
        av_pool = ctx.enter_context(
            tc.tile_pool(name="av", bufs=1, space="PSUM"))

        def emit_load(hp):
            qt = qt_pool.tile([128, S], bf16)
            kt = kt_pool.tile([128, S], bf16)
            nc.gpsimd.dma_start(kt[:, 0:512], kt_d[hp, :, 0:512])
            nc.gpsimd.dma_start(qt[:, 0:512], qt_d[hp, :, 0:512])
            vba = vb_pool.tile([128, NT * 80], bf16, name="vba")
            vbb = vb_pool.tile([128, NT * 80], bf16, name="vbb")
            nc.gpsimd.dma_start(vba[:], vb_d[2 * hp])
            nc.gpsimd.dma_start(vbb[:], vb_d[2 * hp + 1])
            nc.gpsimd.dma_start(kt[:, 512:S], kt_d[hp, :, 512:S])
            nc.gpsimd.dma_start(qt[:, 512:S], qt_d[hp, :, 512:S])
            return qt, kt, vba, vbb

        def emit_epilogue(h, qg, av):
            # av: out^T [80, 512] fp32 PSUM (row 64 = denominator)
            ot = o_pool.tile([80, 512], bf16)
            if epicopy == "act":
                nc.scalar.copy(ot[:], av[:])
            else:
                nc.vector.tensor_copy(ot[:], av[:])
            xt = x_pool.tile([128, 4 * 80], bf16)
            xt3 = xt[:].rearrange("p (t e) -> p t e", e=80)
            for t in range(4):
                nc.sync.dma_start(
                    xt3[:, t, :], ot[:, t * 128:(t + 1) * 128], transpose=True)
            rec = r_pool.tile([128, 4], f32)
            nc.vector.reciprocal(
                rec[:].rearrange("p (t e) -> p t e", e=1), xt3[:, :, 64:65])
            res = res_pool.tile([128, 4 * 64], bf16)
            res3 = res[:].rearrange("p (t d) -> p t d", d=64)
            for t in range(4):
                if epimul == "pool":
                    nc.gpsimd.tensor_scalar_mul(
                        res3[:, t, :], xt3[:, t, 0:64], rec[:, t:t + 1])
                elif epimul == "act" or (epimul == "mix" and t % 2 == 0):
                    nc.scalar.mul(res3[:, t, :], xt3[:, t, 0:64],
                                  rec[:, t:t + 1])
                else:
                    nc.vector.tensor_scalar_mul(
                        res3[:, t, :], xt3[:, t, 0:64], rec[:, t:t + 1])
            nc.sync.dma_start(
                o_d[h, qg * 512:(qg + 1) * 512, :].rearrange(
                    "(t p) d -> p t d", p=128),
                res3,
            )

        def emit_body():
            pending = []
            for hp in range(H_PER // 2):
                qt, kt, vba, vbb = emit_load(hp)
                for qg in range(QG):
                    ava = av_pool.tile([80, 512], f32, name="ava")
                    avb = av_pool.tile([80, 512], f32, name="avb")

                    def emit_av(c, pt_ap):
                        if "noav" in probe and c > 0:
                            return
                        stop = (c == NT - 1) or "noav" in probe
                        nc.tensor.matmul(
                            ava[:],
                            lhsT=vba[:, c * 80:(c + 1) * 80],
                            rhs=pt_ap[:, 0:512],
                            start=(c == 0), stop=stop,
                        )
                        nc.tensor.matmul(
                            avb[:],
                            lhsT=vbb[:, c * 80:(c + 1) * 80],
                            rhs=pt_ap[:, 512:1024],
                            start=(c == 0), stop=stop,
                        )

                    avq = []
                    for c in range(NT):
                        sp = sps_pool.tile([128, 1024], f32)
                        qkw = 128 if "smallqk" in probe else 512
                        for half in range(2):
                            nc.tensor.matmul(
                                sp[:, half * 512:half * 512 + qkw],
                                lhsT=kt[half * 64:(half + 1) * 64,
                                        c * 128:(c + 1) * 128],
                                rhs=qt[half * 64:(half + 1) * 64,
                                       qg * 512:qg * 512 + qkw],
                                start=True,
                                stop=True,
                                tile_position=(half * 64, 0),
                            )
                        pb = pb_pool.tile([128, 1024], bf16)
                        if c in dve_set:
                            nc.vector.tensor_scalar(
                                pb[:].bitcast(i16), sp[:], SCHA, SCHB,
                                Alu.mult, Alu.add)
                        else:
                            w = 256 if "smallexp" in probe else 1024
                            nc.scalar.activation(
                                pb[:, 0:w], sp[:, 0:w],
                                mybir.ActivationFunctionType.Exp,
                                scale=SCALE)
                        if c in (1, 2) and pending:
                            emit_epilogue(*pending.pop(0))
                        if len(avq) == 2:
                            emit_av(*avq.pop(0))
                        avq.append((c, pb[:]))
                    for item in avq:
                        emit_av(*item)
                    avq.clear()
                    pending = [(2 * hp, qg, ava), (2 * hp + 1, qg, avb)]
            for item in pending:
                emit_epilogue(*item)

        if hw_loop_reps > 1:
            with tc.For_i(0, hw_loop_reps):
                for _ in range(hw_unroll):
                    emit_body()
        else:
            for rep in range(reps):
                emit_body()

    nc.compile()
    return nc


def _build_nc7(d_dve=7, schc=0.045, avq=0, probe="", reps=1,
               hw_loop_reps=1, hw_unroll=1):
    """v7: fully row-group-alternating PE stream (microbench-validated: MMs
    at tile_position (0,0)/(64,0) stream CONCURRENTLY ~247ns/pair, while
    same-position MMs serialize at ~424ns each because LDWEIGHTS only pulls
    ahead when the row group differs).

      - QK: head-pair stacked, (0,0)/(64,0) -> sp [128, 1024] = [A | B].
      - AV: each chunk split into two 64-contraction halves, paired ACROSS
        heads so concurrent MMs hit different banks: slot1 = [A_lo@(0,0) ||
        B_hi@(64,0)], slot2 = [A_hi@(64,0) || B_lo@(0,0)]. Each head's
        halves land serially in ONE accumulator bank -> full sums, 2 banks.
      - exp: FD=1024 per chunk, ACT/DVE split by chunk (d_dve of 16 to DVE
        via Schraudolph bf16-bits).
      - Epilogue (on-device, v3-style): bf16 copy -> 4 PE transposes (bf16,
        into a bf16 PSUM tile) -> batched reciprocal -> 4 per-partition
        muls (all-16-bit, 2x DVE mode; mixed ACT/DVE) -> bf16 res -> DMA;
        host upcasts to f32.
    PSUM: sp (2 bufs x 2 banks) + avA + avB + tps (2 bufs x 1) = 8.
    """
    import concourse.tile as tile
    from concourse import bacc, mybir
    from contextlib import ExitStack

    f32 = mybir.dt.float32
    bf16 = mybir.dt.bfloat16
    i16 = mybir.dt.int16
    Alu = mybir.AluOpType

    assert 0 <= d_dve <= 14
    if d_dve == 0:
        dve_set = set()
    elif d_dve == 1:
        dve_set = {8}
    else:
        dve_set = {2 + round(i * 13 / (d_dve - 1)) for i in range(d_dve)}
        j = 2
        while len(dve_set) < d_dve:
            if j not in dve_set:
                dve_set.add(j)
            j += 1
    SCHA = float(SCALE * LOG2E * (1 << 7))
    SCHB = float(127.0 * (1 << 7) - schc * (1 << 7))
    avq_split = bool(avq)

    nc = bacc.Bacc("TRN2", target_bir_lowering=False, debug=False)

    qt_d = nc.dram_tensor("qt", [H_PER // 2, 128, S], bf16,
                          kind="ExternalInput").ap()
    kt_d = nc.dram_tensor("kt", [H_PER // 2, 128, S], bf16,
                          kind="ExternalInput").ap()
    vb_d = nc.dram_tensor("vb", [H_PER, 128, NT * 80], bf16,
                          kind="ExternalInput").ap()
    o_d = nc.dram_tensor("out", [H_PER, S, D], f32, kind="ExternalOutput").ap()

    with tile.TileContext(nc) as tc, ExitStack() as ctx:
        from concourse.masks import make_identity

        qt_pool = ctx.enter_context(tc.tile_pool(name="qt", bufs=2))
        kt_pool = ctx.enter_context(tc.tile_pool(name="kt", bufs=2))
        vb_pool = ctx.enter_context(tc.tile_pool(name="vb", bufs=2))
        pb_pool = ctx.enter_context(tc.tile_pool(name="pb", bufs=6))
        o_pool = ctx.enter_context(tc.tile_pool(name="op", bufs=2))
        r_pool = ctx.enter_context(tc.tile_pool(name="rp", bufs=3))
        res_pool = ctx.enter_context(tc.tile_pool(name="resp", bufs=3))
        id_pool = ctx.enter_context(tc.tile_pool(name="idp", bufs=1))
        sps_pool = ctx.enter_context(
            tc.tile_pool(name="sps", bufs=2, space="PSUM"))
        av_pool = ctx.enter_context(
            tc.tile_pool(name="av", bufs=1, space="PSUM"))
        tps_pool = ctx.enter_context(
            tc.tile_pool(name="tps", bufs=2, space="PSUM"))

        ident = id_pool.tile([128, 128], f32)
        make_identity(nc, ident[:])

        def emit_load(hp):
            qt = qt_pool.tile([128, S], bf16)
            kt = kt_pool.tile([128, S], bf16)
            nc.gpsimd.dma_start(kt[:, 0:512], kt_d[hp, :, 0:512])
            nc.gpsimd.dma_start(qt[:, 0:512], qt_d[hp, :, 0:512])
            vba = vb_pool.tile([128, NT * 80], bf16, name="vba")
            vbb = vb_pool.tile([128, NT * 80], bf16, name="vbb")
            nc.gpsimd.dma_start(vba[:], vb_d[2 * hp])
            nc.gpsimd.dma_start(vbb[:], vb_d[2 * hp + 1])
            nc.gpsimd.dma_start(kt[:, 512:S], kt_d[hp, :, 512:S])
            nc.gpsimd.dma_start(qt[:, 512:S], qt_d[hp, :, 512:S])
            return qt, kt, vba, vbb

        def emit_epilogue(h, qg, av):
            sb = o_pool.tile([80, 512], f32)
            nc.vector.tensor_copy(sb[:], av[:])
            tp = tps_pool.tile([128, 4 * 80], f32)
            tp3 = tp[:].rearrange("p (t e) -> p t e", e=80)
            for t in range(4):
                nc.tensor.transpose(
                    tp[:, t * 80:(t + 1) * 80],
                    sb[:, t * 128:(t + 1) * 128],
                    ident[0:80, 0:80],
                )
            rec = r_pool.tile([128, 4], f32)
            nc.vector.reciprocal(
                rec[:].rearrange("p (t e) -> p t e", e=1), tp3[:, :, 64:65])
            res = res_pool.tile([128, 4 * 64], f32)
            res3 = res[:].rearrange("p (t d) -> p t d", d=64)
            for t in range(4):
                nc.vector.tensor_scalar_mul(
                    res3[:, t, :], tp3[:, t, 0:64], rec[:, t:t + 1])
            nc.sync.dma_start(
                o_d[h, qg * 512:(qg + 1) * 512, :].rearrange(
                    "(t p) d -> p t d", p=128),
                res3,
            )

        def emit_body():
            pending = []
            for hp in range(H_PER // 2):
                qt, kt, vba, vbb = emit_load(hp)
                for qg in range(QG):
                    ava = av_pool.tile([80, 512], f32, name="ava")
                    avb = av_pool.tile([80, 512], f32, name="avb")

                    # AV chunk = four 64-contraction quarter-MMs, emitted in
                    # a row-group-alternating chain interleaved with the QK
                    # pair. Consecutive MMs always differ in row group (so
                    # LDWEIGHTS pulls ahead / streams overlap) AND in target
                    # PSUM bank (so concurrent drains never race); same-bank
                    # MMs are 3 apart, fenced by the same-row-group
                    # serialization in between.
                    def av_mm(av, vb, hf, col, c, start, stop):
                        if "noav" in probe and c > 0:
                            return
                        nc.tensor.matmul(
                            av[:],
                            lhsT=vb[hf * 64:(hf + 1) * 64,
                                    c * 80:(c + 1) * 80],
                            rhs=pt_refs[c][hf * 64:(hf + 1) * 64,
                                           col * 512:(col + 1) * 512],
                            start=start, stop=stop,
                            tile_position=(hf * 64, 0),
                        )

                    def av_full(av, vb, col, c, start, stop):
                        # v3-style full-contraction AV matmul at (0,0)
                        if "noav" in probe and c > 0:
                            return
                        nc.tensor.matmul(
                            av[:],
                            lhsT=vb[:, c * 80:(c + 1) * 80],
                            rhs=pt_refs[c][:, col * 512:(col + 1) * 512],
                            start=start, stop=stop,
                        )

                    pt_refs = {}
                    avq = []
                    for c in range(NT):
                        prev = avq.pop(0) if len(avq) == 2 else None
                        nav = "noav" in probe
                        if prev is not None:
                            pstop = (prev == NT - 1) or nav
                            if avq_split:
                                av_mm(ava, vba, 0, 0, prev,
                                      prev == 0, False)      # A-lo @(0,0)
                                av_mm(avb, vbb, 1, 1, prev,
                                      prev == 0, False)      # B-hi @(64,0)
                            else:
                                # both AV matmuls BEFORE the QK pair: AV uses
                                # all row groups, so placing one between QK_A
                                # and QK_B would fence the pair apart.
                                av_full(ava, vba, 0, prev, prev == 0, pstop)
                                av_full(avb, vbb, 1, prev, prev == 0, pstop)
                        sp = sps_pool.tile([128, 1024], f32)
                        qkw = 128 if "smallqk" in probe else 512
                        nc.tensor.matmul(
                            sp[:, 0:qkw],
                            lhsT=kt[0:64, c * 128:(c + 1) * 128],
                            rhs=qt[0:64, qg * 512:qg * 512 + qkw],
                            start=True, stop=True,
                            tile_position=(0, 0),
                        )
                        if prev is not None and avq_split:
                            pstop = (prev == NT - 1) or nav
                            av_mm(ava, vba, 1, 0, prev,
                                  False, pstop)              # A-hi @(64,0)
                            av_mm(avb, vbb, 0, 1, prev,
                                  False, pstop)              # B-lo @(0,0)
                        nc.tensor.matmul(
                            sp[:, 512:512 + qkw],
                            lhsT=kt[64:128, c * 128:(c + 1) * 128],
                            rhs=qt[64:128, qg * 512:qg * 512 + qkw],
                            start=True, stop=True,
                            tile_position=(64, 0),
                        )
                        pb = pb_pool.tile([128, 1024], bf16)
                        if c in dve_set:
                            nc.vector.tensor_scalar(
                                pb[:].bitcast(i16), sp[:], SCHA, SCHB,
                                Alu.mult, Alu.add)
                        else:
                            w = 256 if "smallexp" in probe else 1024
                            nc.scalar.activation(
                                pb[:, 0:w], sp[:, 0:w],
                                mybir.ActivationFunctionType.Exp,
                                scale=SCALE)
                        if c in (1, 2) and pending:
                            emit_epilogue(*pending.pop(0))
                        pt_refs[c] = pb[:]
                        avq.append(c)
                    for prev in avq:
                        nav = "noav" in probe
                        pstop = (prev == NT - 1) or nav
                        if avq_split:
                            av_mm(ava, vba, 0, 0, prev, prev == 0, False)
                            av_mm(avb, vbb, 1, 1, prev, prev == 0, False)
                            av_mm(ava, vba, 1, 0, prev, False, pstop)
                            av_mm(avb, vbb, 0, 1, prev, False, pstop)
                        else:
                            av_full(ava, vba, 0, prev, prev == 0, pstop)
                            av_full(avb, vbb, 1, prev, prev == 0, pstop)
                    avq.clear()
                    pending = [(2 * hp, qg, ava), (2 * hp + 1, qg, avb)]
            for item in pending:
                emit_epilogue(*item)

        if hw_loop_reps > 1:
            with tc.For_i(0, hw_loop_reps):
                for _ in range(hw_unroll):
                    emit_body()
        else:
            for rep in range(reps):
                emit_body()

    nc.compile()
    return nc


def _cfg():
    import os

    if os.environ.get("ATT_V9", "1") == "1":
        return (
            "v9",
            int(os.environ.get("ATT_DVE", "2")),
            float(os.environ.get("ATT_SCHC", "0.045")),
            os.environ.get("ATT_EPICOPY", "dve"),
            int(os.environ.get("ATT_AVP", "0")),
            os.environ.get("ATT_PROBE", ""),
        )
    if os.environ.get("ATT_V7", "0") == "1":
        return (
            "v7",
            int(os.environ.get("ATT_DVE", "7")),
            float(os.environ.get("ATT_SCHC", "0.045")),
            int(os.environ.get("ATT_AVQ", "0")),
            os.environ.get("ATT_PROBE", ""),
        )
    if os.environ.get("ATT_V6", "0") == "1":
        return (
            "v6",
            int(os.environ.get("ATT_DVE", "8")),
            float(os.environ.get("ATT_SCHC", "0.045")),
            os.environ.get("ATT_EPIMUL", "mix"),
            os.environ.get("ATT_EPICOPY", "dve"),
            os.environ.get("ATT_PROBE", ""),
        )
    if os.environ.get("ATT_V5", "0") == "1":
        return (
            "v5",
            int(os.environ.get("ATT_DVE", "13")),
            float(os.environ.get("ATT_SCHC", "0.045")),
            os.environ.get("ATT_EPIMUL", "act"),
            int(os.environ.get("ATT_SPSB", "2")),
            os.environ.get("ATT_PROBE", ""),
        )
    if os.environ.get("ATT_V4", "0") == "1":
        return (
            "v4",
            int(os.environ.get("ATT_DVE", "6")),
            float(os.environ.get("ATT_SCHC", "0.045")),
            os.environ.get("ATT_EPICOPY", "dve"),
            os.environ.get("ATT_PROBE", ""),
        )
    if os.environ.get("ATT_V3", "1") == "1":
        return (
            "v3",
            int(os.environ.get("ATT_DVE", "2")),
            float(os.environ.get("ATT_SCHC", "0.045")),
            os.environ.get("ATT_EPICOPY", "dve"),
            os.environ.get("ATT_PROBE", ""),
        )
    if os.environ.get("ATT_V2", "1") == "1":
        return (
            "v2",
            int(os.environ.get("ATT_DVE", "4")),
            os.environ.get("ATT_EPICOPY", "dve"),
            float(os.environ.get("ATT_SCHC", "0.045")),
            os.environ.get("ATT_PROBE", ""),
        )
    return (
        os.environ.get("ATT_EPI", "pe"),
        os.environ.get("ATT_QK", "bf16"),
        int(os.environ.get("ATT_GSZ", "2")),
        int(os.environ.get("ATT_SPSB", "3")),
        os.environ.get("ATT_PROBE", ""),
    )


def _build(cfg, **kw):
    if cfg[0] == "v9":
        return _build_nc9(*cfg[1:], **kw)
    if cfg[0] == "v7":
        return _build_nc7(*cfg[1:], **kw)
    if cfg[0] == "v6":
        return _build_nc6(*cfg[1:], **kw)
    if cfg[0] == "v5":
        return _build_nc5(*cfg[1:], **kw)
    if cfg[0] == "v4":
        return _build_nc4(*cfg[1:], **kw)
    if cfg[0] == "v3":
        return _build_nc3(*cfg[1:], **kw)
    if cfg[0] == "v2":
        return _build_nc2(*cfg[1:], **kw)
    return _build_nc(*cfg, **kw)


def _get_nc():
    cfg = _cfg()
    if cfg not in _CACHE:
        _CACHE[cfg] = _build(cfg)
    return _CACHE[cfg]


def _prep_in_maps(Q, K, V):
    import ml_dtypes

    cfg = _cfg()
    is_v2 = cfg[0] in ("v2", "v3", "v4", "v5", "v6", "v7", "v9")
    qk = "bf16" if is_v2 else cfg[1]
    tdt = ml_dtypes.bfloat16 if qk == "bf16" else np.float32
    Qr = np.ascontiguousarray(np.asarray(Q, dtype=np.float32)).reshape(B * H, S, D)
    Kr = np.ascontiguousarray(np.asarray(K, dtype=np.float32)).reshape(B * H, S, D)
    Vr = np.ascontiguousarray(np.asarray(V, dtype=np.float32)).reshape(B * H, S, D)
    # host-side layout prep: [BH, S, D] -> [BH, D, S]
    QT = np.ascontiguousarray(Qr.transpose(0, 2, 1)).astype(tdt)
    KT = np.ascontiguousarray(Kr.transpose(0, 2, 1)).astype(tdt)
    if is_v2:
        # packed V chunks [BH, 128, NT, E] = [V | ones | zeros]
        VE = 68 if cfg[0] == "v5" else 80
        # (v6 uses the classic 80-wide V-weights layout)
        vp = np.zeros((B * H, 128, NT, VE), dtype=np.float32)
        vp[:, :, :, 0:64] = Vr.reshape(B * H, NT, 128, D).transpose(0, 2, 1, 3)
        vp[:, :, :, 64] = 1.0
        vp = vp.reshape(B * H, 128, NT * VE)
        vpb = vp.astype(ml_dtypes.bfloat16)
    in_maps = []
    for c in range(N_CORES):
        sl = slice(c * H_PER, (c + 1) * H_PER)
        if is_v2:
            qtc = np.ascontiguousarray(QT[sl])
            ktc = np.ascontiguousarray(KT[sl])
            if cfg[0] in ("v4", "v5", "v6", "v7"):
                qtc = qtc.reshape(H_PER // 2, 128, S)
                ktc = ktc.reshape(H_PER // 2, 128, S)
            elif cfg[0] == "v9":
                # row-duplicate [64, S] -> [128, S] = [X; X] per head so QK
                # chunk MMs can alternate tile_position (0,0)/(64,0)
                qtc = np.ascontiguousarray(
                    np.concatenate([qtc, qtc], axis=1))
                ktc = np.ascontiguousarray(
                    np.concatenate([ktc, ktc], axis=1))
            m = {
                "qt": qtc,
                "kt": ktc,
                "vb": np.ascontiguousarray(vpb[sl]),
            }
        else:
            m = {
                "qt": np.ascontiguousarray(QT[sl]),
                "kt": np.ascontiguousarray(KT[sl]),
                "v": np.ascontiguousarray(Vr[sl]),
            }
        in_maps.append(m)
    return in_maps


def _gather(results):
    out = np.concatenate([np.asarray(r["out"]) for r in results], axis=0)
    return out.reshape(B, H, S, D).astype(np.float32)


def _numpy_fallback(Q, K, V, mask):
    # generic masked path (not used by the benchmark inputs: mask is all-False)
    Qf = np.asarray(Q, dtype=np.float64)
    Kf = np.asarray(K, dtype=np.float64)
    Vf = np.asarray(V, dtype=np.float64)
    out = np.empty((B, H, S, D), dtype=np.float32)
    for b in range(B):
        for h in range(H):
            s = Qf[b, h] @ Kf[b, h].T
            s = np.where(mask, -1e10, s) / math.sqrt(S)
            s -= s.max(axis=-1, keepdims=True)
            e = np.exp(s)
            p = e / e.sum(axis=-1, keepdims=True)
            out[b, h] = (p @ Vf[b, h]).astype(np.float32)
    return out


def _get_runner():
    """Build the sharded jit callable once; reuse across kernel() calls."""
    key = ("runner",) + _cfg()
    if key in _CACHE:
        return _CACHE[key]
    import jax
    from jax.sharding import Mesh, PartitionSpec, NamedSharding
    from jax.experimental.shard_map import shard_map
    from concourse import bass2jax, mybir
    from concourse.bass2jax import _bass_exec_p, install_neuronx_cc_hook

    nc = _get_nc()
    install_neuronx_cc_hook()
    devices = jax.devices()[:N_CORES]
    assert len(devices) == N_CORES
    mesh = Mesh(np.asarray(devices), ("core",))

    part_name = nc.partition_id_tensor.name if nc.partition_id_tensor else None
    in_names, out_names, out_avals, out_shapes = [], [], [], []
    for alloc in nc.m.functions[0].allocations:
        if not isinstance(alloc, mybir.MemoryLocationSet):
            continue
        name = alloc.memorylocations[0].name
        if alloc.kind == "ExternalInput":
            if name != part_name:
                in_names.append(name)
        elif alloc.kind == "ExternalOutput":
            out_names.append(name)
            shape = tuple(alloc.tensor_shape)
            dtype = mybir.dt.np(alloc.dtype)
            out_avals.append(jax.core.ShapedArray(shape, dtype))
            out_shapes.append((shape, dtype))
    all_names = in_names + out_names + ([part_name] if part_name else [])

    def _body(*args):
        operands = list(args)
        if part_name is not None:
            operands.append(bass2jax.partition_id_tensor())
        return tuple(
            _bass_exec_p.bind(
                *operands,
                out_avals=tuple(out_avals),
                in_names=tuple(all_names),
                out_names=tuple(out_names),
                lowering_input_output_aliases=(),
                sim_require_finite=True,
                sim_require_nnan=True,
                nc=nc,
            )
        )

    nio = len(in_names) + len(out_names)
    fn = jax.jit(
        shard_map(
            _body,
            mesh=mesh,
            in_specs=(PartitionSpec("core"),) * nio,
            out_specs=(PartitionSpec("core"),) * len(out_names),
            check_rep=False,
        ),
        keep_unused=True,
    )
    sh = NamedSharding(mesh, PartitionSpec("core"))

    def run(in_maps):
        import jax as _jax

        concat_in = [
            _jax.device_put(
                np.concatenate(
                    [np.ascontiguousarray(m[nm]) for m in in_maps], axis=0
                ),
                sh,
            )
            for nm in in_names
        ]
        concat_zeros = [
            _jax.device_put(np.zeros((N_CORES * s[0], *s[1:]), dt), sh)
            for (s, dt) in out_shapes
        ]
        outs = fn(*concat_in, *concat_zeros)
        outs = [np.asarray(o) for o in outs]
        return [
            {
                nm: outs[i].reshape(N_CORES, *out_avals[i].shape)[c]
                for i, nm in enumerate(out_names)
            }
            for c in range(N_CORES)
        ]

    _CACHE[key] = run
    return run


def run_on_device(Q, K, V, trace=False, **trace_kwargs):
    """Compile (cached) + run on the 8 cores. Returns (full_output, results)."""
    in_maps = _prep_in_maps(Q, K, V)
    if trace:
        from concourse.bass_utils import run_bass_kernel_spmd

        nc = _get_nc()
        res = run_bass_kernel_spmd(
            nc, in_maps, list(range(N_CORES)), trace=True, **trace_kwargs
        )
        return _gather(res.results), res
    results = _get_runner()(in_maps)
    return _gather(results), None


def kernel(Q, K, V, mask):
    mask = np.asarray(mask)
    if mask.any():
        return _numpy_fallback(Q, K, V, mask)
    out, _ = run_on_device(Q, K, V, trace=False)
    return out

